# revision 24
# baseline (speedup 1.0000x reference)
"""Multi-head attention (B=2, S=2048, D=1024, H=16) on 8 TRN2 NeuronCores.

Sharding: batch x head-group. Core c handles batch c//4 and heads
[4*(c%4), 4*(c%4)+4). Each core computes its heads' Q/K/V projections
(column-parallel), causal attention, and a row-parallel partial of the
output projection. The host sums the 4 partials per batch (all-reduce
done on host during unshard) and adds dense_b.

All streaming data is bf16 (halves HBM traffic and keeps every matmul
at 1 PE-cycle/row regardless of free-size); accumulation stays in fp32
PSUM. On-core dataflow (transposed, feature-major):
  QT = WqT.T @ XqT   [256, 2048]   (bf16 matmuls, K=1024 in 8 chunks)
  KT, V likewise (V in natural [S, 256] layout, + ones column for row sums)
  per head pair, per 512-wide q-block, per 128-wide k-chunk:
    logitsT [sk=128, sq] = KT_chunk.T @ QT_block   (2 heads row-packed
      at tile_position (0,0)/(64,0), shared 2-bank PSUM tile); for
      diagonal chunks only columns [off:512] are computed (causal clip)
    PT = exp(0.125 * logitsT) -> bf16     (ScalarE, PSUM->SBUF)
    diagonal chunks: multiply PT[off:off+128] by 0/1 upper-tri (DVE)
    OT[65, off:512] += V_aug.T @ PT    (V_aug = [V | ones] -> rows 0:64 =
                                        O^T, row 64 = softmax denominator)
  per head: recip = approx(1/sums); R[64,512] = ones x recip (K=1 matmul);
    OT_norm = OT * R; dense partial outT[., q-block] += denseT_h.T @ OT_norm
Causality: fully-masked k-chunks are skipped and diagonal chunks are
column-clipped (~2x less work). A generic path (any mask) adds
maskT * -8e9 to every chunk instead.
"""

import numpy as np
import ml_dtypes
from contextlib import ExitStack

import concourse.tile as tile
from concourse import bacc, mybir
from concourse.bass_utils import run_bass_kernel_spmd

F32 = mybir.dt.float32
BF16 = mybir.dt.bfloat16
AF = mybir.ActivationFunctionType
ADD = mybir.AluOpType.add
MULT = mybir.AluOpType.mult

NPBF = ml_dtypes.bfloat16

B, S, D, H = 2, 2048, 1024, 16
NCORES = 8
HL = 4            # heads per core
DH = D // H       # 64
DLOC = HL * DH    # 256 local feature dims
SBK = 512         # seq block (q)
NSB = S // SBK    # 4
KCH = 128         # k chunk


def _ts(i, n):
    return slice(i * n, (i + 1) * n)


def build(causal=True, with_bq=False, with_bk=False, with_bv=False,
          x_bufs=2, pt_bufs=8, ev_bufs=5, small_bufs=4, ot_bufs=2):
    nc = bacc.Bacc(None, target_bir_lowering=False)

    xqT = nc.dram_tensor("xqT", [D, S], BF16, kind="ExternalInput")
    xkT = nc.dram_tensor("xkT", [D, S], BF16, kind="ExternalInput")
    xvT = nc.dram_tensor("xvT", [D, S], BF16, kind="ExternalInput")
    wqT = nc.dram_tensor("wqT", [D, DLOC], BF16, kind="ExternalInput")
    wkT = nc.dram_tensor("wkT", [D, DLOC], BF16, kind="ExternalInput")
    wvT = nc.dram_tensor("wvT", [D, DLOC], BF16, kind="ExternalInput")
    dnT = nc.dram_tensor("dnT", [DLOC, D], BF16, kind="ExternalInput")
    if not causal:
        mskT = nc.dram_tensor("mskT", [S, S], F32, kind="ExternalInput")
    bq = nc.dram_tensor("bq", [DLOC], BF16, kind="ExternalInput") if with_bq else None
    bk = nc.dram_tensor("bk", [DLOC], BF16, kind="ExternalInput") if with_bk else None
    bv = nc.dram_tensor("bv", [DLOC], BF16, kind="ExternalInput") if with_bv else None
    outT = nc.dram_tensor("outT", [D, S], BF16, kind="ExternalOutput")

    ones512 = nc.inline_tensor(np.ones((1, 512), NPBF), name="ones512")
    ones128p = nc.inline_tensor(np.ones((128, 1), NPBF), name="ones128p")
    ones6464 = nc.inline_tensor(np.ones((128, 64), NPBF), name="ones6464")
    # upper (inclusive) triangle: tri01[r, c] = 1 if r <= c else 0
    tri_np = np.triu(np.ones((KCH, KCH), np.float32)).astype(NPBF)
    tri_in = nc.inline_tensor(tri_np, name="tri01")

    with tile.TileContext(nc) as tc, ExitStack() as ctx:
        pers = ctx.enter_context(tc.tile_pool(name="pers", bufs=1))
        xpool = ctx.enter_context(tc.tile_pool(name="xpool", bufs=x_bufs))
        ptp = ctx.enter_context(tc.tile_pool(name="ptp", bufs=pt_bufs))
        otp = ctx.enter_context(tc.tile_pool(name="otp", bufs=ot_bufs))
        evp = ctx.enter_context(tc.tile_pool(name="evp", bufs=ev_bufs))
        smallp = ctx.enter_context(tc.tile_pool(name="smallp", bufs=small_bufs))
        if not causal:
            mskp = ctx.enter_context(tc.tile_pool(name="mskp", bufs=3))
        mmp = ctx.enter_context(tc.tile_pool(name="mmp", bufs=2, space="PSUM"))
        lp = ctx.enter_context(tc.tile_pool(name="lp", bufs=2, space="PSUM"))
        opp = ctx.enter_context(tc.tile_pool(name="opp", bufs=1, space="PSUM"))

        # ---------- persistent tiles ----------
        wsb = {}
        for wname in ("q", "k", "v"):
            wsb[wname] = pers.tile([128, 8, DLOC], BF16, tag=f"w{wname}",
                                   name=f"w_{wname}")
        dn_sb = pers.tile([128, 2, D], BF16, tag="dn")
        tri01 = pers.tile([KCH, KCH], BF16, tag="tri01")
        ones_r = pers.tile([1, 512], BF16, tag="ones_r")
        onescol = pers.tile([128, 1], BF16, tag="onescol")
        ones64 = pers.tile([128, 64], BF16, tag="ones64")

        QT_sb = pers.tile([128, 2, S], BF16, tag="QT")
        KT_sb = pers.tile([128, 2, S], BF16, tag="KT")
        V_sb = pers.tile([128, S // KCH, HL, DH + 1], BF16, tag="V")

        bsb = {}
        for name, dram in (("q", bq), ("k", bk), ("v", bv)):
            if dram is not None:
                t = pers.tile([1, DLOC], BF16, tag=f"b{name}")
                nc.sync.dma_start(out=t, in_=dram[None, :])
                bsb[name] = t

        xdram = {"q": xqT, "k": xkT, "v": xvT}
        wdram = {"q": wqT, "k": wkT, "v": wvT}
        xt_pend = {}

        def load_x(j, js):
            xt = {}
            for xname in ("q", "k", "v"):
                srcr = xdram[xname].rearrange("(c p) s -> p c s", p=128)
                t = xpool.tile([128, 8, SBK], BF16, tag=f"x{xname}",
                               name=f"x_{xname}_{j}")
                # two-way split so the first consuming matmuls can start
                # halfway through the block's transfer
                nc.sync.dma_start(out=t[:, 0:4, :], in_=srcr[:, 0:4, js])
                nc.sync.dma_start(out=t[:, 4:8, :], in_=srcr[:, 4:8, js])
                xt[xname] = t
            return xt

        js0 = _ts(0, SBK)
        for xname in ("q", "k", "v"):
            wr = wdram[xname].rearrange("(c p) m -> p c m", p=128)
            if xname == "q":
                # 2-way split: the first projection matmuls only need the
                # low-kc half of wq
                nc.sync.dma_start(out=wsb[xname][:, 0:4, :], in_=wr[:, 0:4, :])
                nc.sync.dma_start(out=wsb[xname][:, 4:8, :], in_=wr[:, 4:8, :])
            else:
                nc.sync.dma_start(out=wsb[xname], in_=wr)
            srcr = xdram[xname].rearrange("(c p) s -> p c s", p=128)
            t = xpool.tile([128, 8, SBK], BF16, tag=f"x{xname}",
                           name=f"x_{xname}_0")
            # 4-way split: the startup is DMA-bound, so let the first
            # projection matmuls start a quarter of the way in
            for q4 in range(4):
                nc.sync.dma_start(out=t[:, _ts(q4, 2), :],
                                  in_=srcr[:, _ts(q4, 2), js0])
            xt_pend[0] = xt_pend.get(0, {})
            xt_pend[0][xname] = t

        # consts after the first x block: tiny, and none is needed before
        # the first V-projection evac (~12us in)
        nc.sync.dma_start(out=ones_r, in_=ones512[:, :])
        nc.sync.dma_start(out=onescol, in_=ones128p[:, :])
        nc.sync.dma_start(out=ones64, in_=ones6464[:, :])
        # ones column of V_aug (softmax denominator trick)
        nc.vector.tensor_copy(
            V_sb[:, :, :, DH:DH + 1],
            onescol[:, None, None, :].broadcast_to([128, S // KCH, HL, 1]),
        )

        outT_r = outT.rearrange("(c p) s -> p c s", p=128)

        # round-robin copy engines for PSUM evacuation (ACT is reserved for
        # exp during attention; phase-A-only copies may use it)
        def copier(engines=("vector",), _state={}):
            k = engines
            i = _state.get(k, 0)
            _state[k] = i + 1
            eng = getattr(nc, engines[i % len(engines)])

            def cp(out, in_):
                if hasattr(eng, "tensor_copy"):
                    eng.tensor_copy(out, in_)
                else:
                    eng.copy(out=out, in_=in_)
            return type("C", (), {"tensor_copy": staticmethod(cp)})

        def phase_A_groups(j, js, xt, evac=("vector",)):
            # ---------- projections for s-block j, as 8 independent
            # matmul-group thunks so they can be interleaved into phase B ----
            def qk_group(bname, dst, mc):
                def emit():
                    ps = mmp.tile([128, 512], F32, tag="mm")
                    has_b = bname in bsb
                    for kc in range(8):
                        nc.tensor.matmul(
                            ps[:, :],
                            lhsT=wsb[bname][:, kc, _ts(mc, 128)],
                            rhs=xt[bname][:, kc, :],
                            start=(kc == 0), stop=(kc == 7 and not has_b),
                        )
                    if has_b:
                        nc.tensor.matmul(
                            ps[:, :], lhsT=bsb[bname][0:1, _ts(mc, 128)],
                            rhs=ones_r[0:1, 0:SBK], start=False, stop=True,
                        )
                    copier(evac).tensor_copy(dst[:, mc, js], ps)
                return emit

            def v_group(sc):
                def emit():
                    ps = mmp.tile([128, 512], F32, tag="mm")
                    has_b = "v" in bsb
                    for kc in range(8):
                        nc.tensor.matmul(
                            ps[:, 0:DLOC],
                            lhsT=xt["v"][:, kc, _ts(sc, 128)],
                            rhs=wsb["v"][:, kc, :],
                            start=(kc == 0), stop=(kc == 7 and not has_b),
                        )
                    if has_b:
                        nc.tensor.matmul(
                            ps[:, 0:DLOC], lhsT=ones_r[0:1, 0:128],
                            rhs=bsb["v"][0:1, :], start=False, stop=True,
                        )
                    copier(evac).tensor_copy(
                        V_sb[:, j * 4 + sc, :, 0:DH],
                        ps[:, 0:DLOC].rearrange("p (h d) -> p h d", h=HL),
                    )
                return emit

            return ([qk_group(b, d, mc) for b, d in (("q", QT_sb), ("k", KT_sb))
                     for mc in range(2)] + [v_group(sc) for sc in range(4)])

        def denorm_thunks(j, pc, O, OTs, sm):
            # softmax denominators: sums row (lane 64, staged to SBUF by the
            # caller right after the AV flush), broadcast to partitions 0:64
            # via K=1 matmul, then a single fused divide straight out of the
            # O accumulator in PSUM. Drip-fed into the NEXT attention loop so
            # the PE never blocks on the cross-engine chain.
            otpair = otp.tile([128, 512], BF16, tag=f"otp{pc}",
                              name=f"otp_{j}_{pc}")
            OTs[pc] = otpair
            st = {}

            def t_bcast():
                for i in range(2):
                    Sps = mmp.tile([128, 512], F32, tag="mm",
                                   name=f"sps_{j}_{pc}_{i}")
                    nc.tensor.matmul(
                        Sps[0:64, :], lhsT=ones64[64:65, 0:64],
                        rhs=sm[i][64:65, :],
                        start=True, stop=True, tile_position=(64, 0),
                    )
                    st[i] = Sps

            def t_rc():
                # DVE ops may read at most ONE operand from PSUM, so take the
                # reciprocal into SBUF first; the multiply then pairs the O
                # accumulator (PSUM) with rc (SBUF)
                for i in range(2):
                    rc = smallp.tile([64, 512], F32, tag="rc",
                                     name=f"rc_{j}_{pc}_{i}")
                    nc.vector.reciprocal_approx_fast(out=rc, in_=st[i][0:64, :])
                    st[i] = rc

            def t_mul0():
                nc.vector.tensor_tensor(
                    out=otpair[0:64, :], in0=O[0][0:64, :],
                    in1=st[0], op=MULT)

            def t_mul1():
                ot_tmp = smallp.tile([64, 512], BF16, tag="ott",
                                     name=f"ott_{j}_{pc}")
                nc.vector.tensor_tensor(
                    out=ot_tmp, in0=O[1][0:64, :],
                    in1=st[1], op=MULT)
                # partition shift 0:64 -> 64:128 (SBUF->SBUF DMA)
                nc.sync.dma_start(out=otpair[64:128, :], in_=ot_tmp[:, :])

            return [t_bcast, t_rc, t_mul0, t_mul1]

        def dense_thunks(j, js, OTs):
            def grp(mc):
                def emit():
                    if j == NSB - 1 and mc % 2:
                        # tail: the attention loops are over, so the L pool's
                        # banks are free — rotate through them too to double
                        # the dense pipeline depth
                        dps = lp.tile([128, 2, SBK], F32, tag="L",
                                      name=f"dpsl_{j}_{mc}")[:, 0, :]
                    else:
                        dps = mmp.tile([128, 512], F32, tag="mm",
                                       name=f"dps_{j}_{mc}")
                    for pc in range(2):
                        nc.tensor.matmul(
                            dps[:, :], lhsT=dn_sb[:, pc, _ts(mc, 128)],
                            rhs=OTs[pc][:, :], start=(pc == 0), stop=(pc == 1),
                        )
                    ev = evp.tile([128, 512], BF16, tag="ev",
                                  name=f"ev_{j}_{mc}")
                    copier().tensor_copy(ev, dps)
                    nc.sync.dma_start(out=outT_r[:, mc, js], in_=ev)
                return emit
            return [grp(mc) for mc in range(8)]

        def phase_B(j, js, fill):
            # ---------- attention + dense for q-block j ----------
            # `fill`: queue of thunks (phase-A groups, previous block's
            # denorm + dense), one emitted per kc iteration right after the
            # logits matmuls, where the PE would otherwise wait on the exp.
            nkc = (j + 1) * 4 if causal else S // KCH
            OTs = [None, None]
            for pc in range(2):
                O = [
                    opp.tile([65, 512], F32, tag=f"o{i}", name=f"O_{j}_{pc}_{i}")
                    for i in range(2)
                ]
                pend = []  # software pipeline: AV trails logits by PD kc's

                def emit_av(kc, off, last, PT):
                    for i in range(2):
                        nc.tensor.matmul(
                            O[i][0:65, off:SBK],
                            lhsT=V_sb[:, kc, 2 * pc + i, :],
                            rhs=PT[:, i, off:SBK],
                            start=(kc == 0), stop=last,
                            skip_group_check=True,
                        )

                for kc in range(nkc):
                    diag = causal and kc >= 4 * j
                    off = (kc - 4 * j) * KCH if diag else 0
                    L = lp.tile([128, 2, SBK], F32, tag="L")
                    for i in range(2):
                        nc.tensor.matmul(
                            L[:, i, off:SBK],
                            lhsT=KT_sb[_ts(i, 64), pc, _ts(kc, KCH)],
                            rhs=QT_sb[_ts(i, 64), pc,
                                      j * SBK + off:(j + 1) * SBK],
                            start=True, stop=True,
                            tile_position=(64 * i, 0),
                        )
                    if fill:
                        fill.pop(0)()
                    if not causal:
                        mk = mskp.tile([128, SBK], F32, tag="mk")
                        nc.sync.dma_start(out=mk, in_=mskT[_ts(kc, KCH), js])
                        nc.vector.tensor_tensor(
                            out=L[:, :, :], in0=L[:, :, :],
                            in1=mk[:, None, :].broadcast_to([128, 2, SBK]),
                            op=ADD,
                        )
                    PT = ptp.tile([128, 2, SBK], BF16, tag="PT")
                    nc.scalar.activation(
                        out=PT[:, :, off:SBK], in_=L[:, :, off:SBK],
                        func=AF.Exp, scale=0.125)
                    if diag:
                        # triangle mask as 0/1 multiply (off the exp edge)
                        nc.vector.tensor_tensor(
                            out=PT[:, :, off:off + KCH],
                            in0=PT[:, :, off:off + KCH],
                            in1=tri01[:, None, :].broadcast_to([128, 2, KCH]),
                            op=MULT,
                        )
                    pend.append((kc, off, kc == nkc - 1, PT))
                    if len(pend) > 3:
                        emit_av(*pend.pop(0))
                for p in pend:
                    emit_av(*p)
                # stage the sums rows to SBUF immediately (frees the O
                # accumulators' WAR hazard early; PE isn't involved)
                sm = []
                for i in range(2):
                    t = smallp.tile([65, 512], BF16, tag="sm",
                                    name=f"sm_{j}_{pc}_{i}")
                    nc.scalar.copy(out=t[64:65, :], in_=O[i][64:65, :])
                    sm.append(t)
                # denorm gets priority over queued dense/A thunks: the next
                # attention round's first AV blocks on the O-buffer WAR until
                # the divides have read O
                fill[:0] = denorm_thunks(j, pc, O, OTs, sm)
            fill.extend(dense_thunks(j, js, OTs))

        # schedule: A0, A1 up front (PE runway; ACT free, so evacs may use
        # it), then B(j) with A(j+2)'s groups and B(j-1)'s denorm + dense
        # drip-fed into the exp-paced attention loops.
        xts = {0: xt_pend.pop(0)}
        for j in range(NSB):
            if j + 1 not in xts and j + 1 < NSB:
                xts[j + 1] = load_x(j + 1, _ts(j + 1, SBK))
            if j < 2:
                for g in phase_A_groups(j, _ts(j, SBK), xts.pop(j),
                                        evac=("scalar", "vector")):
                    g()
                if j == 0:
                    # deprioritized loads: queued behind the first x-blocks;
                    # still well ahead of their first consumers
                    nc.sync.dma_start(out=tri01, in_=tri_in[:, :])
                    nc.sync.dma_start(
                        out=dn_sb,
                        in_=dnT.rearrange("(c p) n -> p c n", p=128))
        fill = []
        for j in range(NSB):
            if j + 2 < NSB:
                fill.extend(phase_A_groups(j + 2, _ts(j + 2, SBK),
                                           xts.pop(j + 2)))
            phase_B(j, _ts(j, SBK), fill)
        while fill:
            fill.pop(0)()

    nc.finalize()
    return nc


_CACHE = {}


def _get_nc(causal, with_bq, with_bk, with_bv):
    key = (causal, with_bq, with_bk, with_bv)
    if key not in _CACHE:
        _CACHE[key] = build(causal, with_bq, with_bk, with_bv)
    return _CACHE[key]


def _bf(a):
    return np.ascontiguousarray(a).astype(NPBF)


def _prep_in_maps(query, key_, value, mask2d, causal, wq_w, wk_w, wv_w, dense_w,
                  wq_b, wk_b, wv_b, with_bq, with_bk, with_bv):
    in_maps = []
    xT = {}
    for b in range(B):
        xT[b] = (_bf(query[b].T), _bf(key_[b].T), _bf(value[b].T))
    mskT = None if causal else np.ascontiguousarray(mask2d.T * np.float32(-8e9))
    for c in range(NCORES):
        b, g = divmod(c, 4)
        sl = _ts(g, DLOC)
        m = {
            "xqT": xT[b][0], "xkT": xT[b][1], "xvT": xT[b][2],
            "wqT": _bf(wq_w[sl].T),
            "wkT": _bf(wk_w[sl].T),
            "wvT": _bf(wv_w[sl].T),
            "dnT": _bf(dense_w[:, sl].T),
        }
        if not causal:
            m["mskT"] = mskT
        if with_bq:
            m["bq"] = _bf(wq_b[sl])
        if with_bk:
            m["bk"] = _bf(wk_b[sl])
        if with_bv:
            m["bv"] = _bf(wv_b[sl])
        in_maps.append(m)
    return in_maps


def _run(in_maps, causal, with_bq, with_bk, with_bv, **kw):
    nc = _get_nc(causal, with_bq, with_bk, with_bv)
    return run_bass_kernel_spmd(nc, in_maps, core_ids=list(range(NCORES)), **kw)


def kernel(query, key_, value, mask, wq_w, wq_b, wk_w, wk_b, wv_w, wv_b,
           dense_w, dense_b, _profile_kw=None):
    query = np.asarray(query, np.float32)
    key_ = np.asarray(key_, np.float32)
    value = np.asarray(value, np.float32)
    mask2d = np.asarray(mask, np.float32).reshape(S, S)
    wq_w = np.asarray(wq_w, np.float32)
    wk_w = np.asarray(wk_w, np.float32)
    wv_w = np.asarray(wv_w, np.float32)
    dense_w = np.asarray(dense_w, np.float32)
    wq_b = np.asarray(wq_b, np.float32)
    wk_b = np.asarray(wk_b, np.float32)
    wv_b = np.asarray(wv_b, np.float32)
    dense_b = np.asarray(dense_b, np.float32)

    causal = bool(np.array_equal(mask2d, np.triu(np.ones((S, S), np.float32), k=1)))
    with_bq = bool(np.any(wq_b))
    with_bk = bool(np.any(wk_b))
    with_bv = bool(np.any(wv_b))

    in_maps = _prep_in_maps(query, key_, value, mask2d, causal, wq_w, wk_w, wv_w,
                            dense_w, wq_b, wk_b, wv_b, with_bq, with_bk, with_bv)
    res = _run(in_maps, causal, with_bq, with_bk, with_bv, **(_profile_kw or {}))

    out = np.empty((B, S, D), np.float32)
    for b in range(B):
        acc = res.results[4 * b]["outT"].astype(np.float32).copy()
        for g in range(1, 4):
            acc += res.results[4 * b + g]["outT"]
        out[b] = acc.T + dense_b[None, :]
    if _profile_kw:
        return out, res
    return out


# revision 48
# speedup vs baseline: 1.0397x; 1.0397x over previous
"""Multi-head attention (B=2, S=2048, D=1024, H=16) on 8 TRN2 NeuronCores.

Sharding: batch x head-group. Core c handles batch c//4 and heads
[4*(c%4), 4*(c%4)+4). Each core computes its heads' Q/K/V projections
(column-parallel), causal attention, and a row-parallel partial of the
output projection. The host sums the 4 partials per batch (all-reduce
done on host during unshard) and adds dense_b.

All streaming data is bf16 (halves HBM traffic and keeps every matmul
at 1 PE-cycle/row regardless of free-size); accumulation stays in fp32
PSUM. On-core dataflow (transposed, feature-major):
  QT = WqT.T @ XqT   [256, 2048]   (bf16 matmuls, K=1024 in 8 chunks)
  KT, V likewise (V in natural [S, 256] layout, + ones column for row sums)
  per head pair, per 512-wide q-block, per 128-wide k-chunk:
    logitsT [sk=128, sq] = KT_chunk.T @ QT_block   (2 heads row-packed
      at tile_position (0,0)/(64,0), shared 2-bank PSUM tile); for
      diagonal chunks only columns [off:512] are computed (causal clip)
    PT = exp(0.125 * logitsT) -> bf16     (ScalarE, PSUM->SBUF)
    diagonal chunks: multiply PT[off:off+128] by 0/1 upper-tri (DVE)
    OT[65, off:512] += V_aug.T @ PT    (V_aug = [V | ones] -> rows 0:64 =
                                        O^T, row 64 = softmax denominator)
  per head: recip = approx(1/sums); R[64,512] = ones x recip (K=1 matmul);
    OT_norm = OT * R; dense partial outT[., q-block] += denseT_h.T @ OT_norm
Causality: fully-masked k-chunks are skipped and diagonal chunks are
column-clipped (~2x less work). A generic path (any mask) adds
maskT * -8e9 to every chunk instead.
"""

import numpy as np
import ml_dtypes
from contextlib import ExitStack

import concourse.tile as tile
from concourse import bacc, mybir
from concourse.bass_utils import run_bass_kernel_spmd

F32 = mybir.dt.float32
BF16 = mybir.dt.bfloat16
AF = mybir.ActivationFunctionType
ADD = mybir.AluOpType.add
MULT = mybir.AluOpType.mult

NPBF = ml_dtypes.bfloat16

B, S, D, H = 2, 2048, 1024, 16
NCORES = 8
HL = 4            # heads per core
DH = D // H       # 64
DLOC = HL * DH    # 256 local feature dims
SBK = 512         # seq block (q)
NSB = S // SBK    # 4
KCH = 128         # k chunk


def _ts(i, n):
    return slice(i * n, (i + 1) * n)


def build(causal=True, with_bq=False, with_bk=False, with_bv=False,
          x_bufs=2, pt_bufs=12, ev_bufs=5, small_bufs=4, ot_bufs=2):
    nc = bacc.Bacc(None, target_bir_lowering=False)

    xqT = nc.dram_tensor("xqT", [D, S], BF16, kind="ExternalInput")
    xkT = nc.dram_tensor("xkT", [D, S], BF16, kind="ExternalInput")
    xvT = nc.dram_tensor("xvT", [D, S], BF16, kind="ExternalInput")
    wqT = nc.dram_tensor("wqT", [D, DLOC], BF16, kind="ExternalInput")
    wkT = nc.dram_tensor("wkT", [D, DLOC], BF16, kind="ExternalInput")
    wvT = nc.dram_tensor("wvT", [D, DLOC], BF16, kind="ExternalInput")
    dnT = nc.dram_tensor("dnT", [DLOC, D], BF16, kind="ExternalInput")
    if not causal:
        mskT = nc.dram_tensor("mskT", [S, S], F32, kind="ExternalInput")
    bq = nc.dram_tensor("bq", [DLOC], BF16, kind="ExternalInput") if with_bq else None
    bk = nc.dram_tensor("bk", [DLOC], BF16, kind="ExternalInput") if with_bk else None
    bv = nc.dram_tensor("bv", [DLOC], BF16, kind="ExternalInput") if with_bv else None
    outT = nc.dram_tensor("outT", [D, S], BF16, kind="ExternalOutput")

    ones512 = nc.inline_tensor(np.ones((1, 512), NPBF), name="ones512")
    ones128p = nc.inline_tensor(np.ones((128, 1), NPBF), name="ones128p")
    ones6464 = nc.inline_tensor(np.ones((128, 64), NPBF), name="ones6464")
    # upper (inclusive) triangle: tri01[r, c] = 1 if r <= c else 0
    tri_np = np.triu(np.ones((KCH, KCH), np.float32)).astype(NPBF)
    tri_in = nc.inline_tensor(tri_np, name="tri01")

    with tile.TileContext(nc) as tc, ExitStack() as ctx:
        pers = ctx.enter_context(tc.tile_pool(name="pers", bufs=1))
        xpool = ctx.enter_context(tc.tile_pool(name="xpool", bufs=x_bufs))
        ptp = ctx.enter_context(tc.tile_pool(name="ptp", bufs=pt_bufs))
        otp = ctx.enter_context(tc.tile_pool(name="otp", bufs=ot_bufs))
        evp = ctx.enter_context(tc.tile_pool(name="evp", bufs=ev_bufs))
        smallp = ctx.enter_context(tc.tile_pool(name="smallp", bufs=small_bufs))
        if not causal:
            mskp = ctx.enter_context(tc.tile_pool(name="mskp", bufs=3))
        mmp = ctx.enter_context(tc.tile_pool(name="mmp", bufs=2, space="PSUM"))
        lp = ctx.enter_context(tc.tile_pool(name="lp", bufs=2, space="PSUM"))
        opp = ctx.enter_context(tc.tile_pool(name="opp", bufs=1, space="PSUM"))

        # ---------- persistent tiles ----------
        wsb = {}
        for wname in ("q", "k", "v"):
            wsb[wname] = pers.tile([128, 8, DLOC], BF16, tag=f"w{wname}",
                                   name=f"w_{wname}")
        dn_sb = pers.tile([128, 2, D], BF16, tag="dn")
        tri01 = pers.tile([KCH, KCH], BF16, tag="tri01")
        ones_r = pers.tile([1, 512], BF16, tag="ones_r")
        onescol = pers.tile([128, 1], BF16, tag="onescol")
        ones64 = pers.tile([128, 64], BF16, tag="ones64")

        QT_sb = pers.tile([128, 2, S], BF16, tag="QT")
        KT_sb = pers.tile([128, 2, S], BF16, tag="KT")
        V_sb = pers.tile([128, S // KCH, HL, DH + 1], BF16, tag="V")

        bsb = {}
        for name, dram in (("q", bq), ("k", bk), ("v", bv)):
            if dram is not None:
                t = pers.tile([1, DLOC], BF16, tag=f"b{name}")
                nc.sync.dma_start(out=t, in_=dram[None, :])
                bsb[name] = t

        xdram = {"q": xqT, "k": xkT, "v": xvT}
        wdram = {"q": wqT, "k": wkT, "v": wvT}
        xt_pend = {}

        def load_x(j, js):
            xt = {}
            for xname in ("q", "k", "v"):
                srcr = xdram[xname].rearrange("(c p) s -> p c s", p=128)
                t = xpool.tile([128, 8, SBK], BF16, tag=f"x{xname}",
                               name=f"x_{xname}_{j}")
                # two-way split so the first consuming matmuls can start
                # halfway through the block's transfer
                nc.sync.dma_start(out=t[:, 0:4, :], in_=srcr[:, 0:4, js])
                nc.sync.dma_start(out=t[:, 4:8, :], in_=srcr[:, 4:8, js])
                xt[xname] = t
            return xt

        js0 = _ts(0, SBK)
        for xname in ("q", "k", "v"):
            wr = wdram[xname].rearrange("(c p) m -> p c m", p=128)
            if xname == "q":
                # 2-way split: the first projection matmuls only need the
                # low-kc half of wq
                nc.sync.dma_start(out=wsb[xname][:, 0:4, :], in_=wr[:, 0:4, :])
                nc.sync.dma_start(out=wsb[xname][:, 4:8, :], in_=wr[:, 4:8, :])
            else:
                nc.sync.dma_start(out=wsb[xname], in_=wr)
            srcr = xdram[xname].rearrange("(c p) s -> p c s", p=128)
            t = xpool.tile([128, 8, SBK], BF16, tag=f"x{xname}",
                           name=f"x_{xname}_0")
            # 4-way split: the startup is DMA-bound, so let the first
            # projection matmuls start a quarter of the way in
            for q4 in range(4):
                nc.sync.dma_start(out=t[:, _ts(q4, 2), :],
                                  in_=srcr[:, _ts(q4, 2), js0])
            xt_pend[0] = xt_pend.get(0, {})
            xt_pend[0][xname] = t

        # consts after the first x block: tiny, and none is needed before
        # the first V-projection evac (~12us in)
        nc.sync.dma_start(out=ones_r, in_=ones512[:, :])
        nc.sync.dma_start(out=onescol, in_=ones128p[:, :])
        nc.sync.dma_start(out=ones64, in_=ones6464[:, :])
        # ones column of V_aug (softmax denominator trick)
        nc.vector.tensor_copy(
            V_sb[:, :, :, DH:DH + 1],
            onescol[:, None, None, :].broadcast_to([128, S // KCH, HL, 1]),
        )

        outT_r = outT.rearrange("(c p) s -> p c s", p=128)

        # round-robin copy engines for PSUM evacuation (ACT is reserved for
        # exp during attention; phase-A-only copies may use it)
        def copier(engines=("vector",), _state={}):
            k = engines
            i = _state.get(k, 0)
            _state[k] = i + 1
            eng = getattr(nc, engines[i % len(engines)])

            def cp(out, in_):
                if hasattr(eng, "tensor_copy"):
                    eng.tensor_copy(out, in_)
                else:
                    eng.copy(out=out, in_=in_)
            return type("C", (), {"tensor_copy": staticmethod(cp)})

        def phase_A_groups(j, js, xt, evac=("vector",)):
            # ---------- projections for s-block j, as 8 independent
            # matmul-group thunks so they can be interleaved into phase B ----
            def qk_group(bname, dst, mc):
                def emit():
                    ps = mmp.tile([128, 512], F32, tag="mm")
                    has_b = bname in bsb
                    for kc in range(8):
                        nc.tensor.matmul(
                            ps[:, :],
                            lhsT=wsb[bname][:, kc, _ts(mc, 128)],
                            rhs=xt[bname][:, kc, :],
                            start=(kc == 0), stop=(kc == 7 and not has_b),
                        )
                    if has_b:
                        nc.tensor.matmul(
                            ps[:, :], lhsT=bsb[bname][0:1, _ts(mc, 128)],
                            rhs=ones_r[0:1, 0:SBK], start=False, stop=True,
                        )
                    copier(evac).tensor_copy(dst[:, mc, js], ps)
                return emit

            def v_group(sc):
                def emit():
                    ps = mmp.tile([128, 512], F32, tag="mm")
                    has_b = "v" in bsb
                    for kc in range(8):
                        nc.tensor.matmul(
                            ps[:, 0:DLOC],
                            lhsT=xt["v"][:, kc, _ts(sc, 128)],
                            rhs=wsb["v"][:, kc, :],
                            start=(kc == 0), stop=(kc == 7 and not has_b),
                        )
                    if has_b:
                        nc.tensor.matmul(
                            ps[:, 0:DLOC], lhsT=ones_r[0:1, 0:128],
                            rhs=bsb["v"][0:1, :], start=False, stop=True,
                        )
                    copier(evac).tensor_copy(
                        V_sb[:, j * 4 + sc, :, 0:DH],
                        ps[:, 0:DLOC].rearrange("p (h d) -> p h d", h=HL),
                    )
                return emit

            return ([qk_group(b, d, mc) for b, d in (("q", QT_sb), ("k", KT_sb))
                     for mc in range(2)] + [v_group(sc) for sc in range(4)])

        def denorm_thunks(j, pc, O, OTs, sm):
            # softmax denominators: sums row (lane 64, staged to SBUF by the
            # caller right after the AV flush), broadcast to partitions 0:64
            # via K=1 matmul, then a single fused divide straight out of the
            # O accumulator in PSUM. Drip-fed into the NEXT attention loop so
            # the PE never blocks on the cross-engine chain.
            otpair = otp.tile([128, 512], BF16, tag=f"otp{pc}",
                              name=f"otp_{j}_{pc}")
            OTs[pc] = otpair
            st = {}

            def t_bcast():
                for i in range(2):
                    Sps = mmp.tile([128, 512], F32, tag="mm",
                                   name=f"sps_{j}_{pc}_{i}")
                    nc.tensor.matmul(
                        Sps[0:64, :], lhsT=ones64[64:65, 0:64],
                        rhs=sm[i][64:65, :],
                        start=True, stop=True, tile_position=(64, 0),
                    )
                    st[i] = Sps

            def t_rc():
                # DVE ops may read at most ONE operand from PSUM, so take the
                # reciprocal into SBUF first; the multiply then pairs the O
                # accumulator (PSUM) with rc (SBUF)
                for i in range(2):
                    rc = smallp.tile([64, 512], F32, tag="rc",
                                     name=f"rc_{j}_{pc}_{i}")
                    nc.vector.reciprocal_approx_fast(out=rc, in_=st[i][0:64, :])
                    st[i] = rc

            def t_mul0():
                nc.vector.tensor_tensor(
                    out=otpair[0:64, :], in0=O[0][0:64, :],
                    in1=st[0], op=MULT)

            def t_mul1():
                ot_tmp = smallp.tile([64, 512], BF16, tag="ott",
                                     name=f"ott_{j}_{pc}")
                nc.vector.tensor_tensor(
                    out=ot_tmp, in0=O[1][0:64, :],
                    in1=st[1], op=MULT)
                # partition shift 0:64 -> 64:128 (gpsimd can remap
                # partitions SBUF->SBUF without the DMA latency chain)
                nc.gpsimd.tensor_copy(otpair[64:128, :], ot_tmp[:, :])

            return [t_bcast, t_rc, t_mul0, t_mul1]

        def dense_thunks(j, js, OTs):
            box = {}

            def grp(mc):
                def emit():
                    if j == NSB - 1 and mc % 2:
                        # tail: the attention loops are over, so the L pool's
                        # banks are free — rotate through them too to double
                        # the dense pipeline depth
                        dps = lp.tile([128, 2, SBK], F32, tag="L",
                                      name=f"dpsl_{j}_{mc}")[:, 0, :]
                    else:
                        dps = mmp.tile([128, 512], F32, tag="mm",
                                       name=f"dps_{j}_{mc}")
                    for pc in range(2):
                        nc.tensor.matmul(
                            dps[:, :], lhsT=dn_sb[:, pc, _ts(mc, 128)],
                            rhs=OTs[pc][:, :], start=(pc == 0), stop=(pc == 1),
                        )
                    # pair consecutive mc's into one ev tile / one out-DMA
                    # (halves the 625ns-per-DMA HWDGE serialization)
                    if mc % 2 == 0:
                        box["ev"] = evp.tile([128, 2, 512], BF16, tag="ev",
                                             name=f"ev_{j}_{mc}")
                    ev = box["ev"]
                    if j == NSB - 1:
                        # tail: exps are done, ACT is free — split each evac
                        # across DVE and ACT to halve the drain
                        nc.vector.tensor_copy(ev[:, mc % 2, 0:256],
                                              dps[:, 0:256])
                        nc.scalar.copy(out=ev[:, mc % 2, 256:512],
                                       in_=dps[:, 256:512])
                    else:
                        copier().tensor_copy(ev[:, mc % 2, :], dps)
                    if mc % 2:
                        nc.sync.dma_start(
                            out=outT_r[:, mc - 1:mc + 1, js],
                            in_=ev)
                return emit
            return [grp(mc) for mc in range(8)]

        def phase_B(j, js, fill):
            # ---------- attention + dense for q-block j ----------
            # `fill`: queue of thunks (phase-A groups, previous block's
            # denorm + dense), one emitted per kc iteration right after the
            # logits matmuls, where the PE would otherwise wait on the exp.
            nkc = (j + 1) * 4 if causal else S // KCH
            OTs = [None, None]
            for pc in range(2):
                O = [
                    opp.tile([65, 512], F32, tag=f"o{i}", name=f"O_{j}_{pc}_{i}")
                    for i in range(2)
                ]
                pend = []  # software pipeline: AV trails logits by PD kc's

                def emit_av(kc, off, last, PT):
                    for i in range(2):
                        nc.tensor.matmul(
                            O[i][0:65, off:SBK],
                            lhsT=V_sb[:, kc, 2 * pc + i, :],
                            rhs=PT[:, i, off:SBK],
                            start=(kc == 0), stop=last,
                            skip_group_check=True,
                        )

                for kc in range(nkc):
                    diag = causal and kc >= 4 * j
                    off = (kc - 4 * j) * KCH if diag else 0
                    L = lp.tile([128, 2, SBK], F32, tag="L")
                    for i in range(2):
                        nc.tensor.matmul(
                            L[:, i, off:SBK],
                            lhsT=KT_sb[_ts(i, 64), pc, _ts(kc, KCH)],
                            rhs=QT_sb[_ts(i, 64), pc,
                                      j * SBK + off:(j + 1) * SBK],
                            start=True, stop=True,
                            tile_position=(64 * i, 0),
                        )
                    if fill and (j > 0 or (pc * nkc + kc) % 2):
                        fill.pop(0)()
                    if not causal:
                        mk = mskp.tile([128, SBK], F32, tag="mk")
                        nc.sync.dma_start(out=mk, in_=mskT[_ts(kc, KCH), js])
                        nc.vector.tensor_tensor(
                            out=L[:, :, :], in0=L[:, :, :],
                            in1=mk[:, None, :].broadcast_to([128, 2, SBK]),
                            op=ADD,
                        )
                    PT = ptp.tile([128, 2, SBK], BF16, tag="PT")
                    nc.scalar.activation(
                        out=PT[:, :, off:SBK], in_=L[:, :, off:SBK],
                        func=AF.Exp, scale=0.125)
                    if diag:
                        # triangle mask as 0/1 multiply (off the exp edge)
                        nc.vector.tensor_tensor(
                            out=PT[:, :, off:off + KCH],
                            in0=PT[:, :, off:off + KCH],
                            in1=tri01[:, None, :].broadcast_to([128, 2, KCH]),
                            op=MULT,
                        )
                    pend.append((kc, off, kc == nkc - 1, PT))
                    if len(pend) > 4:
                        emit_av(*pend.pop(0))
                for p in pend:
                    emit_av(*p)
                # stage the sums rows to SBUF immediately (frees the O
                # accumulators' WAR hazard early; PE isn't involved)
                sm = []
                for i in range(2):
                    t = smallp.tile([65, 512], BF16, tag="sm",
                                    name=f"sm_{j}_{pc}_{i}")
                    copier().tensor_copy(t[64:65, :], O[i][64:65, :])
                    sm.append(t)
                # denorm gets priority over queued dense/A thunks EXCEPT one
                # older thunk kept in front: it buys the sums-staging copies a
                # full iteration of slack before the broadcast matmul reads
                # them (the next round's first AV still unblocks early enough)
                head, rest = fill[:2], fill[2:]
                fill[:] = head + denorm_thunks(j, pc, O, OTs, sm) + rest
            fill.extend(dense_thunks(j, js, OTs))

        # schedule: A0, A1 up front (PE runway; ACT free, so evacs may use
        # it), then B(j) with A(j+2)'s groups and B(j-1)'s denorm + dense
        # drip-fed into the exp-paced attention loops.
        xts = {0: xt_pend.pop(0)}
        for j in range(NSB):
            if j + 1 not in xts and j + 1 < NSB:
                xts[j + 1] = load_x(j + 1, _ts(j + 1, SBK))
            if j < 2:
                for g in phase_A_groups(j, _ts(j, SBK), xts.pop(j),
                                        evac=("scalar", "vector")):
                    g()
                if j == 0:
                    # deprioritized loads: queued behind the first x-blocks;
                    # still well ahead of their first consumers
                    nc.sync.dma_start(out=tri01, in_=tri_in[:, :])
                    nc.sync.dma_start(
                        out=dn_sb,
                        in_=dnT.rearrange("(c p) n -> p c n", p=128))
        fill = []
        for j in range(NSB):
            if j + 2 < NSB:
                fill.extend(phase_A_groups(j + 2, _ts(j + 2, SBK),
                                           xts.pop(j + 2)))
            phase_B(j, _ts(j, SBK), fill)
        while fill:
            fill.pop(0)()

    nc.finalize()
    return nc


_CACHE = {}


def _get_nc(causal, with_bq, with_bk, with_bv):
    key = (causal, with_bq, with_bk, with_bv)
    if key not in _CACHE:
        _CACHE[key] = build(causal, with_bq, with_bk, with_bv)
    return _CACHE[key]


def _bf(a):
    return np.ascontiguousarray(a).astype(NPBF)


def _prep_in_maps(query, key_, value, mask2d, causal, wq_w, wk_w, wv_w, dense_w,
                  wq_b, wk_b, wv_b, with_bq, with_bk, with_bv):
    in_maps = []
    xT = {}
    for b in range(B):
        xT[b] = (_bf(query[b].T), _bf(key_[b].T), _bf(value[b].T))
    mskT = None if causal else np.ascontiguousarray(mask2d.T * np.float32(-8e9))
    for c in range(NCORES):
        b, g = divmod(c, 4)
        sl = _ts(g, DLOC)
        m = {
            "xqT": xT[b][0], "xkT": xT[b][1], "xvT": xT[b][2],
            "wqT": _bf(wq_w[sl].T),
            "wkT": _bf(wk_w[sl].T),
            "wvT": _bf(wv_w[sl].T),
            "dnT": _bf(dense_w[:, sl].T),
        }
        if not causal:
            m["mskT"] = mskT
        if with_bq:
            m["bq"] = _bf(wq_b[sl])
        if with_bk:
            m["bk"] = _bf(wk_b[sl])
        if with_bv:
            m["bv"] = _bf(wv_b[sl])
        in_maps.append(m)
    return in_maps


def _run(in_maps, causal, with_bq, with_bk, with_bv, **kw):
    nc = _get_nc(causal, with_bq, with_bk, with_bv)
    return run_bass_kernel_spmd(nc, in_maps, core_ids=list(range(NCORES)), **kw)


def kernel(query, key_, value, mask, wq_w, wq_b, wk_w, wk_b, wv_w, wv_b,
           dense_w, dense_b, _profile_kw=None):
    query = np.asarray(query, np.float32)
    key_ = np.asarray(key_, np.float32)
    value = np.asarray(value, np.float32)
    mask2d = np.asarray(mask, np.float32).reshape(S, S)
    wq_w = np.asarray(wq_w, np.float32)
    wk_w = np.asarray(wk_w, np.float32)
    wv_w = np.asarray(wv_w, np.float32)
    dense_w = np.asarray(dense_w, np.float32)
    wq_b = np.asarray(wq_b, np.float32)
    wk_b = np.asarray(wk_b, np.float32)
    wv_b = np.asarray(wv_b, np.float32)
    dense_b = np.asarray(dense_b, np.float32)

    causal = bool(np.array_equal(mask2d, np.triu(np.ones((S, S), np.float32), k=1)))
    with_bq = bool(np.any(wq_b))
    with_bk = bool(np.any(wk_b))
    with_bv = bool(np.any(wv_b))

    in_maps = _prep_in_maps(query, key_, value, mask2d, causal, wq_w, wk_w, wv_w,
                            dense_w, wq_b, wk_b, wv_b, with_bq, with_bk, with_bv)
    res = _run(in_maps, causal, with_bq, with_bk, with_bv, **(_profile_kw or {}))

    out = np.empty((B, S, D), np.float32)
    for b in range(B):
        acc = res.results[4 * b]["outT"].astype(np.float32).copy()
        for g in range(1, 4):
            acc += res.results[4 * b + g]["outT"]
        out[b] = acc.T + dense_b[None, :]
    if _profile_kw:
        return out, res
    return out


# revision 55
# speedup vs baseline: 1.0447x; 1.0048x over previous
"""Multi-head attention (B=2, S=2048, D=1024, H=16) on 8 TRN2 NeuronCores.

Sharding: batch x head-group. Core c handles batch c//4 and heads
[4*(c%4), 4*(c%4)+4). Each core computes its heads' Q/K/V projections
(column-parallel), causal attention, and a row-parallel partial of the
output projection. The host sums the 4 bf16 partials per batch
(all-reduce done on host during unshard) and adds dense_b.

All streaming data is bf16 (halves HBM traffic and keeps every matmul at
1 PE-cycle/row regardless of free-size); accumulation stays in fp32
PSUM. On-core dataflow (transposed, feature-major):
  QT = WqT.T @ XqT   [256, 2048]   (bf16 matmuls, K=1024 in 8 chunks)
  KT, V likewise (V in natural [S, 256] layout, + ones column for row sums)
  per head pair, per 512-wide q-block, per 128-wide k-chunk:
    logitsT [sk=128, sq] = KT_chunk.T @ QT_block   (2 heads row-packed
      at tile_position (0,0)/(64,0), shared 2-bank PSUM tile); for
      diagonal chunks only columns [off:512] are computed (causal clip)
    PT = exp(0.125 * logitsT) -> bf16     (ScalarE, PSUM->SBUF)
    diagonal chunks: multiply PT[off:off+128] by 0/1 upper-tri (DVE)
    OT[65, off:512] += V_aug.T @ PT    (V_aug = [V | ones] -> rows 0:64 =
                                        O^T, row 64 = softmax denominator)
  per head: sums broadcast to rows 0:64 via K=1 matmul; rc = 1/sums (DVE
    reciprocal into SBUF - DVE ops may read only one PSUM operand);
    OT_norm = OT * rc (head-odd shifted to partitions 64:128 by GPSIMD);
    dense partial outT[., q-block] += denseT.T @ OT_norm, evacuated in
    mc-pairs sharing one HBM DMA
Causality: fully-masked k-chunks are skipped and diagonal chunks are
column-clipped (~2x less work). A generic path (any mask) adds
maskT * -8e9 to every chunk instead.

Schedule: the attention loop is exp-paced (ScalarE ~1040ns per k-chunk
vs ~850ns of PE work), so everything else is drip-fed into it as "fill"
thunks consumed one per iteration: the projections for block j+2, and
the previous block's softmax-denominator chain and dense groups. AV
matmuls trail their logits by 4 chunks (software pipeline) so they never
wait on the exp; the O-accumulator reuse hazard is resolved by giving
the denorm chain queue priority (behind a 2-thunk head that buys the
sums staging some slack).
"""

import numpy as np
import ml_dtypes
from contextlib import ExitStack

import concourse.tile as tile
from concourse import bacc, mybir
from concourse.bass_utils import run_bass_kernel_spmd

F32 = mybir.dt.float32
BF16 = mybir.dt.bfloat16
AF = mybir.ActivationFunctionType
ADD = mybir.AluOpType.add
MULT = mybir.AluOpType.mult

NPBF = ml_dtypes.bfloat16

B, S, D, H = 2, 2048, 1024, 16
NCORES = 8
HL = 4            # heads per core
DH = D // H       # 64
DLOC = HL * DH    # 256 local feature dims
SBK = 512         # seq block (q)
NSB = S // SBK    # 4
KCH = 128         # k chunk


def _ts(i, n):
    return slice(i * n, (i + 1) * n)


def build(causal=True, with_bq=False, with_bk=False, with_bv=False,
          x_bufs=2, pt_bufs=12, ev_bufs=5, small_bufs=4, ot_bufs=2):
    nc = bacc.Bacc(None, target_bir_lowering=False)

    xqT = nc.dram_tensor("xqT", [D, S], BF16, kind="ExternalInput")
    xkT = nc.dram_tensor("xkT", [D, S], BF16, kind="ExternalInput")
    xvT = nc.dram_tensor("xvT", [D, S], BF16, kind="ExternalInput")
    wqT = nc.dram_tensor("wqT", [D, DLOC], BF16, kind="ExternalInput")
    wkT = nc.dram_tensor("wkT", [D, DLOC], BF16, kind="ExternalInput")
    wvT = nc.dram_tensor("wvT", [D, DLOC], BF16, kind="ExternalInput")
    dnT = nc.dram_tensor("dnT", [DLOC, D], BF16, kind="ExternalInput")
    if not causal:
        mskT = nc.dram_tensor("mskT", [S, S], F32, kind="ExternalInput")
    bq = nc.dram_tensor("bq", [DLOC], BF16, kind="ExternalInput") if with_bq else None
    bk = nc.dram_tensor("bk", [DLOC], BF16, kind="ExternalInput") if with_bk else None
    bv = nc.dram_tensor("bv", [DLOC], BF16, kind="ExternalInput") if with_bv else None
    outT = nc.dram_tensor("outT", [D, S], BF16, kind="ExternalOutput")

    ones512 = nc.inline_tensor(np.ones((1, 512), NPBF), name="ones512")
    ones128p = nc.inline_tensor(np.ones((128, 1), NPBF), name="ones128p")
    ones6464 = nc.inline_tensor(np.ones((128, 64), NPBF), name="ones6464")
    # upper (inclusive) triangle: tri01[r, c] = 1 if r <= c else 0
    tri_np = np.triu(np.ones((KCH, KCH), np.float32)).astype(NPBF)
    tri_in = nc.inline_tensor(tri_np, name="tri01")

    with tile.TileContext(nc) as tc, ExitStack() as ctx:
        pers = ctx.enter_context(tc.tile_pool(name="pers", bufs=1))
        xpool = ctx.enter_context(tc.tile_pool(name="xpool", bufs=x_bufs))
        ptp = ctx.enter_context(tc.tile_pool(name="ptp", bufs=pt_bufs))
        otp = ctx.enter_context(tc.tile_pool(name="otp", bufs=ot_bufs))
        evp = ctx.enter_context(tc.tile_pool(name="evp", bufs=ev_bufs))
        smallp = ctx.enter_context(tc.tile_pool(name="smallp", bufs=small_bufs))
        if not causal:
            mskp = ctx.enter_context(tc.tile_pool(name="mskp", bufs=3))
        mmp = ctx.enter_context(tc.tile_pool(name="mmp", bufs=2, space="PSUM"))
        lp = ctx.enter_context(tc.tile_pool(name="lp", bufs=2, space="PSUM"))
        opp = ctx.enter_context(tc.tile_pool(name="opp", bufs=1, space="PSUM"))

        # ---------- persistent tiles ----------
        wsb = {}
        for wname in ("q", "k", "v"):
            wsb[wname] = pers.tile([128, 8, DLOC], BF16, tag=f"w{wname}",
                                   name=f"w_{wname}")
        dn_sb = pers.tile([128, 2, D], BF16, tag="dn")
        tri01 = pers.tile([KCH, KCH], BF16, tag="tri01")
        ones_r = pers.tile([1, 512], BF16, tag="ones_r")
        onescol = pers.tile([128, 1], BF16, tag="onescol")
        ones64 = pers.tile([128, 64], BF16, tag="ones64")

        QT_sb = pers.tile([128, 2, S], BF16, tag="QT")
        KT_sb = pers.tile([128, 2, S], BF16, tag="KT")
        V_sb = pers.tile([128, S // KCH, HL, DH + 1], BF16, tag="V")

        bsb = {}
        for name, dram in (("q", bq), ("k", bk), ("v", bv)):
            if dram is not None:
                t = pers.tile([1, DLOC], BF16, tag=f"b{name}")
                nc.sync.dma_start(out=t, in_=dram[None, :])
                bsb[name] = t

        xdram = {"q": xqT, "k": xkT, "v": xvT}
        wdram = {"q": wqT, "k": wkT, "v": wvT}
        xt_pend = {}

        def load_x(j, js):
            xt = {}
            for xname in ("q", "k", "v"):
                srcr = xdram[xname].rearrange("(c p) s -> p c s", p=128)
                t = xpool.tile([128, 8, SBK], BF16, tag=f"x{xname}",
                               name=f"x_{xname}_{j}")
                # two-way split so the first consuming matmuls can start
                # halfway through the block's transfer
                nc.sync.dma_start(out=t[:, 0:4, :], in_=srcr[:, 0:4, js])
                nc.sync.dma_start(out=t[:, 4:8, :], in_=srcr[:, 4:8, js])
                xt[xname] = t
            return xt

        js0 = _ts(0, SBK)
        for xname in ("q", "k", "v"):
            wr = wdram[xname].rearrange("(c p) m -> p c m", p=128)
            if xname == "q":
                # 2-way split: the first projection matmuls only need the
                # low-kc half of wq
                nc.sync.dma_start(out=wsb[xname][:, 0:4, :], in_=wr[:, 0:4, :])
                nc.sync.dma_start(out=wsb[xname][:, 4:8, :], in_=wr[:, 4:8, :])
            else:
                nc.sync.dma_start(out=wsb[xname], in_=wr)
            srcr = xdram[xname].rearrange("(c p) s -> p c s", p=128)
            t = xpool.tile([128, 8, SBK], BF16, tag=f"x{xname}",
                           name=f"x_{xname}_0")
            # 4-way split: the startup is DMA-bound, so let the first
            # projection matmuls start a quarter of the way in
            for q4 in range(4):
                nc.sync.dma_start(out=t[:, _ts(q4, 2), :],
                                  in_=srcr[:, _ts(q4, 2), js0])
            xt_pend[0] = xt_pend.get(0, {})
            xt_pend[0][xname] = t

        # consts after the first x block: tiny, and none is needed before
        # the first V-projection evac (~12us in)
        nc.sync.dma_start(out=ones_r, in_=ones512[:, :])
        nc.sync.dma_start(out=onescol, in_=ones128p[:, :])
        nc.sync.dma_start(out=ones64, in_=ones6464[:, :])
        # ones column of V_aug (softmax denominator trick)
        nc.vector.tensor_copy(
            V_sb[:, :, :, DH:DH + 1],
            onescol[:, None, None, :].broadcast_to([128, S // KCH, HL, 1]),
        )

        outT_r = outT.rearrange("(c p) s -> p c s", p=128)

        # round-robin copy engines for PSUM evacuation (ACT is reserved for
        # exp during attention; phase-A-only copies may use it)
        def copier(engines=("vector",), _state={}):
            k = engines
            i = _state.get(k, 0)
            _state[k] = i + 1
            eng = getattr(nc, engines[i % len(engines)])

            def cp(out, in_):
                if hasattr(eng, "tensor_copy"):
                    eng.tensor_copy(out, in_)
                else:
                    eng.copy(out=out, in_=in_)
            return type("C", (), {"tensor_copy": staticmethod(cp)})

        def phase_A_groups(j, js, xt, evac=("vector",)):
            # ---------- projections for s-block j, as 8 independent
            # matmul-group thunks so they can be interleaved into phase B ----
            def qk_group(bname, dst, mc):
                def emit():
                    ps = mmp.tile([128, 512], F32, tag="mm")
                    has_b = bname in bsb
                    for kc in range(8):
                        nc.tensor.matmul(
                            ps[:, :],
                            lhsT=wsb[bname][:, kc, _ts(mc, 128)],
                            rhs=xt[bname][:, kc, :],
                            start=(kc == 0), stop=(kc == 7 and not has_b),
                        )
                    if has_b:
                        nc.tensor.matmul(
                            ps[:, :], lhsT=bsb[bname][0:1, _ts(mc, 128)],
                            rhs=ones_r[0:1, 0:SBK], start=False, stop=True,
                        )
                    copier(evac).tensor_copy(dst[:, mc, js], ps)
                return emit

            def v_group(sc):
                def emit():
                    ps = mmp.tile([128, 512], F32, tag="mm")
                    has_b = "v" in bsb
                    for kc in range(8):
                        nc.tensor.matmul(
                            ps[:, 0:DLOC],
                            lhsT=xt["v"][:, kc, _ts(sc, 128)],
                            rhs=wsb["v"][:, kc, :],
                            start=(kc == 0), stop=(kc == 7 and not has_b),
                        )
                    if has_b:
                        nc.tensor.matmul(
                            ps[:, 0:DLOC], lhsT=ones_r[0:1, 0:128],
                            rhs=bsb["v"][0:1, :], start=False, stop=True,
                        )
                    copier(evac).tensor_copy(
                        V_sb[:, j * 4 + sc, :, 0:DH],
                        ps[:, 0:DLOC].rearrange("p (h d) -> p h d", h=HL),
                    )
                return emit

            return ([qk_group(b, d, mc) for b, d in (("q", QT_sb), ("k", KT_sb))
                     for mc in range(2)] + [v_group(sc) for sc in range(4)])

        def denorm_thunks(j, pc, O, OTs, sm):
            # softmax denominators: sums row (lane 64, staged to SBUF by the
            # caller right after the AV flush), broadcast to partitions 0:64
            # via K=1 matmul, then a single fused divide straight out of the
            # O accumulator in PSUM. Drip-fed into the NEXT attention loop so
            # the PE never blocks on the cross-engine chain.
            otpair = otp.tile([128, 512], BF16, tag=f"otp{pc}",
                              name=f"otp_{j}_{pc}")
            OTs[pc] = otpair
            st = {}

            def t_bcast():
                for i in range(2):
                    Sps = mmp.tile([128, 512], F32, tag="mm",
                                   name=f"sps_{j}_{pc}_{i}")
                    nc.tensor.matmul(
                        Sps[0:64, :], lhsT=ones64[64:65, 0:64],
                        rhs=sm[i][64:65, :],
                        start=True, stop=True, tile_position=(64, 0),
                    )
                    st[i] = Sps

            def t_rc():
                # DVE ops may read at most ONE operand from PSUM, so take the
                # reciprocal into SBUF first; the multiply then pairs the O
                # accumulator (PSUM) with rc (SBUF)
                for i in range(2):
                    rc = smallp.tile([64, 512], F32, tag="rc",
                                     name=f"rc_{j}_{pc}_{i}")
                    nc.vector.reciprocal_approx_fast(out=rc, in_=st[i][0:64, :])
                    st[i] = rc

            def t_mul0():
                nc.vector.tensor_tensor(
                    out=otpair[0:64, :], in0=O[0][0:64, :],
                    in1=st[0], op=MULT)

            def t_mul1():
                ot_tmp = smallp.tile([64, 512], BF16, tag="ott",
                                     name=f"ott_{j}_{pc}")
                nc.vector.tensor_tensor(
                    out=ot_tmp, in0=O[1][0:64, :],
                    in1=st[1], op=MULT)
                # partition shift 0:64 -> 64:128 (gpsimd can remap
                # partitions SBUF->SBUF without the DMA latency chain)
                nc.gpsimd.tensor_copy(otpair[64:128, :], ot_tmp[:, :])

            return [t_bcast, t_rc, t_mul0, t_mul1]

        def dense_thunks(j, js, OTs):
            box = {}

            def grp(mc):
                def emit():
                    if j == NSB - 1 and mc % 2:
                        # tail: the attention loops are over, so the L pool's
                        # banks are free — rotate through them too to double
                        # the dense pipeline depth
                        dps = lp.tile([128, 2, SBK], F32, tag="L",
                                      name=f"dpsl_{j}_{mc}")[:, 0, :]
                    else:
                        dps = mmp.tile([128, 512], F32, tag="mm",
                                       name=f"dps_{j}_{mc}")
                    for pc in range(2):
                        nc.tensor.matmul(
                            dps[:, :], lhsT=dn_sb[:, pc, _ts(mc, 128)],
                            rhs=OTs[pc][:, :], start=(pc == 0), stop=(pc == 1),
                        )
                    # pair consecutive mc's into one ev tile / one out-DMA
                    # (halves the 625ns-per-DMA HWDGE serialization)
                    if mc % 2 == 0:
                        box["ev"] = evp.tile([128, 2, 512], BF16, tag="ev",
                                             name=f"ev_{j}_{mc}")
                    ev = box["ev"]
                    if j == NSB - 1:
                        # tail: exps are done, ACT is free — split each evac
                        # across DVE and ACT to halve the drain
                        nc.vector.tensor_copy(ev[:, mc % 2, 0:256],
                                              dps[:, 0:256])
                        nc.scalar.copy(out=ev[:, mc % 2, 256:512],
                                       in_=dps[:, 256:512])
                    else:
                        copier().tensor_copy(ev[:, mc % 2, :], dps)
                    if mc % 2:
                        nc.sync.dma_start(
                            out=outT_r[:, mc - 1:mc + 1, js],
                            in_=ev)
                return emit
            return [grp(mc) for mc in range(8)]

        def phase_B(j, js, fill):
            # ---------- attention + dense for q-block j ----------
            # `fill`: queue of thunks (phase-A groups, previous block's
            # denorm + dense), one emitted per kc iteration right after the
            # logits matmuls, where the PE would otherwise wait on the exp.
            nkc = (j + 1) * 4 if causal else S // KCH
            OTs = [None, None]
            for pc in range(2):
                O = [
                    opp.tile([65, 512], F32, tag=f"o{i}", name=f"O_{j}_{pc}_{i}")
                    for i in range(2)
                ]
                pend = []  # software pipeline: AV trails logits by PD kc's

                def emit_av(kc, off, last, PT):
                    for i in range(2):
                        nc.tensor.matmul(
                            O[i][0:65, off:SBK],
                            lhsT=V_sb[:, kc, 2 * pc + i, :],
                            rhs=PT[:, i, off:SBK],
                            start=(kc == 0), stop=last,
                            skip_group_check=True,
                        )

                for kc in range(nkc):
                    diag = causal and kc >= 4 * j
                    off = (kc - 4 * j) * KCH if diag else 0
                    L = lp.tile([128, 2, SBK], F32, tag="L")
                    for i in range(2):
                        nc.tensor.matmul(
                            L[:, i, off:SBK],
                            lhsT=KT_sb[_ts(i, 64), pc, _ts(kc, KCH)],
                            rhs=QT_sb[_ts(i, 64), pc,
                                      j * SBK + off:(j + 1) * SBK],
                            start=True, stop=True,
                            tile_position=(64 * i, 0),
                        )
                    if fill and not (j == NSB - 1 and pc == 0
                                     and kc % 2 and kc > 4):
                        fill.pop(0)()
                    if not causal:
                        mk = mskp.tile([128, SBK], F32, tag="mk")
                        nc.sync.dma_start(out=mk, in_=mskT[_ts(kc, KCH), js])
                        nc.vector.tensor_tensor(
                            out=L[:, :, :], in0=L[:, :, :],
                            in1=mk[:, None, :].broadcast_to([128, 2, SBK]),
                            op=ADD,
                        )
                    PT = ptp.tile([128, 2, SBK], BF16, tag="PT")
                    nc.scalar.activation(
                        out=PT[:, :, off:SBK], in_=L[:, :, off:SBK],
                        func=AF.Exp, scale=0.125)
                    if diag:
                        # triangle mask as 0/1 multiply (off the exp edge)
                        nc.vector.tensor_tensor(
                            out=PT[:, :, off:off + KCH],
                            in0=PT[:, :, off:off + KCH],
                            in1=tri01[:, None, :].broadcast_to([128, 2, KCH]),
                            op=MULT,
                        )
                    pend.append((kc, off, kc == nkc - 1, PT))
                    if len(pend) > 4:
                        emit_av(*pend.pop(0))
                for p in pend:
                    # absorb each flushed AV's exp-wait with a leftover fill
                    if j == NSB - 1 and fill:
                        fill.pop(0)()
                    emit_av(*p)
                # stage the sums rows to SBUF immediately (frees the O
                # accumulators' WAR hazard early; PE isn't involved)
                sm = []
                for i in range(2):
                    t = smallp.tile([65, 512], BF16, tag="sm",
                                    name=f"sm_{j}_{pc}_{i}")
                    if j == NSB - 1 and pc == 1:
                        # tail: DVE is busy draining dense evacs; ACT is free
                        nc.scalar.copy(out=t[64:65, :], in_=O[i][64:65, :])
                    else:
                        copier().tensor_copy(t[64:65, :], O[i][64:65, :])
                    sm.append(t)
                # denorm gets priority over queued dense/A thunks EXCEPT one
                # older thunk kept in front: it buys the sums-staging copies a
                # full iteration of slack before the broadcast matmul reads
                # them (the next round's first AV still unblocks early enough)
                head, rest = fill[:2], fill[2:]
                fill[:] = head + denorm_thunks(j, pc, O, OTs, sm) + rest
            fill.extend(dense_thunks(j, js, OTs))

        # schedule: A0, A1 up front (PE runway; ACT free, so evacs may use
        # it), then B(j) with A(j+2)'s groups and B(j-1)'s denorm + dense
        # drip-fed into the exp-paced attention loops.
        xts = {0: xt_pend.pop(0)}
        for j in range(NSB):
            if j + 1 not in xts and j + 1 < NSB:
                xts[j + 1] = load_x(j + 1, _ts(j + 1, SBK))
            if j < 2:
                for g in phase_A_groups(j, _ts(j, SBK), xts.pop(j),
                                        evac=("scalar", "vector")):
                    g()
                if j == 0:
                    # deprioritized loads: queued behind the first x-blocks;
                    # still well ahead of their first consumers
                    nc.sync.dma_start(out=tri01, in_=tri_in[:, :])
                    nc.sync.dma_start(
                        out=dn_sb,
                        in_=dnT.rearrange("(c p) n -> p c n", p=128))
        fill = []
        for j in range(NSB):
            if j + 2 < NSB:
                fill.extend(phase_A_groups(j + 2, _ts(j + 2, SBK),
                                           xts.pop(j + 2)))
            phase_B(j, _ts(j, SBK), fill)
        while fill:
            fill.pop(0)()

    nc.finalize()
    return nc


_CACHE = {}


def _get_nc(causal, with_bq, with_bk, with_bv):
    key = (causal, with_bq, with_bk, with_bv)
    if key not in _CACHE:
        _CACHE[key] = build(causal, with_bq, with_bk, with_bv)
    return _CACHE[key]


def _bf(a):
    return np.ascontiguousarray(a).astype(NPBF)


def _prep_in_maps(query, key_, value, mask2d, causal, wq_w, wk_w, wv_w, dense_w,
                  wq_b, wk_b, wv_b, with_bq, with_bk, with_bv):
    in_maps = []
    xT = {}
    for b in range(B):
        xT[b] = (_bf(query[b].T), _bf(key_[b].T), _bf(value[b].T))
    mskT = None if causal else np.ascontiguousarray(mask2d.T * np.float32(-8e9))
    for c in range(NCORES):
        b, g = divmod(c, 4)
        sl = _ts(g, DLOC)
        m = {
            "xqT": xT[b][0], "xkT": xT[b][1], "xvT": xT[b][2],
            "wqT": _bf(wq_w[sl].T),
            "wkT": _bf(wk_w[sl].T),
            "wvT": _bf(wv_w[sl].T),
            "dnT": _bf(dense_w[:, sl].T),
        }
        if not causal:
            m["mskT"] = mskT
        if with_bq:
            m["bq"] = _bf(wq_b[sl])
        if with_bk:
            m["bk"] = _bf(wk_b[sl])
        if with_bv:
            m["bv"] = _bf(wv_b[sl])
        in_maps.append(m)
    return in_maps


def _run(in_maps, causal, with_bq, with_bk, with_bv, **kw):
    nc = _get_nc(causal, with_bq, with_bk, with_bv)
    return run_bass_kernel_spmd(nc, in_maps, core_ids=list(range(NCORES)), **kw)


def kernel(query, key_, value, mask, wq_w, wq_b, wk_w, wk_b, wv_w, wv_b,
           dense_w, dense_b, _profile_kw=None):
    query = np.asarray(query, np.float32)
    key_ = np.asarray(key_, np.float32)
    value = np.asarray(value, np.float32)
    mask2d = np.asarray(mask, np.float32).reshape(S, S)
    wq_w = np.asarray(wq_w, np.float32)
    wk_w = np.asarray(wk_w, np.float32)
    wv_w = np.asarray(wv_w, np.float32)
    dense_w = np.asarray(dense_w, np.float32)
    wq_b = np.asarray(wq_b, np.float32)
    wk_b = np.asarray(wk_b, np.float32)
    wv_b = np.asarray(wv_b, np.float32)
    dense_b = np.asarray(dense_b, np.float32)

    causal = bool(np.array_equal(mask2d, np.triu(np.ones((S, S), np.float32), k=1)))
    with_bq = bool(np.any(wq_b))
    with_bk = bool(np.any(wk_b))
    with_bv = bool(np.any(wv_b))

    in_maps = _prep_in_maps(query, key_, value, mask2d, causal, wq_w, wk_w, wv_w,
                            dense_w, wq_b, wk_b, wv_b, with_bq, with_bk, with_bv)
    res = _run(in_maps, causal, with_bq, with_bk, with_bv, **(_profile_kw or {}))

    out = np.empty((B, S, D), np.float32)
    for b in range(B):
        acc = res.results[4 * b]["outT"].astype(np.float32).copy()
        for g in range(1, 4):
            acc += res.results[4 * b + g]["outT"]
        out[b] = acc.T + dense_b[None, :]
    if _profile_kw:
        return out, res
    return out


# revision 62
# speedup vs baseline: 1.0478x; 1.0029x over previous
"""Multi-head attention (B=2, S=2048, D=1024, H=16) on 8 TRN2 NeuronCores.

Sharding: batch x head-group. Core c handles batch c//4 and heads
[4*(c%4), 4*(c%4)+4). Each core computes its heads' Q/K/V projections
(column-parallel), causal attention, and a row-parallel partial of the
output projection. The host sums the 4 bf16 partials per batch
(all-reduce done on host during unshard) and adds dense_b.

All streaming data is bf16 (halves HBM traffic and keeps every matmul at
1 PE-cycle/row regardless of free-size); accumulation stays in fp32
PSUM. On-core dataflow (transposed, feature-major):
  QT = WqT.T @ XqT   [256, 2048]   (bf16 matmuls, K=1024 in 8 chunks)
  KT, V likewise (V in natural [S, 256] layout, + ones column for row sums)
  per head pair, per 512-wide q-block, per 128-wide k-chunk:
    logitsT [sk=128, sq] = KT_chunk.T @ QT_block   (2 heads row-packed
      at tile_position (0,0)/(64,0), shared 2-bank PSUM tile); for
      diagonal chunks only columns [off:512] are computed (causal clip)
    PT = exp(0.125 * logitsT) -> bf16     (ScalarE, PSUM->SBUF)
    diagonal chunks: multiply PT[off:off+128] by 0/1 upper-tri (DVE)
    OT[65, off:512] += V_aug.T @ PT    (V_aug = [V | ones] -> rows 0:64 =
                                        O^T, row 64 = softmax denominator)
  per head: sums broadcast to rows 0:64 via K=1 matmul; rc = 1/sums (DVE
    reciprocal into SBUF - DVE ops may read only one PSUM operand);
    OT_norm = OT * rc (head-odd shifted to partitions 64:128 by GPSIMD);
    dense partial outT[., q-block] += denseT.T @ OT_norm, evacuated in
    mc-pairs sharing one HBM DMA
Causality: fully-masked k-chunks are skipped and diagonal chunks are
column-clipped (~2x less work). A generic path (any mask) adds
maskT * -8e9 to every chunk instead.

Schedule: the attention loop is exp-paced (ScalarE ~1040ns per k-chunk
vs ~850ns of PE work), so everything else is drip-fed into it as "fill"
thunks consumed one per iteration: the projections for block j+2, and
the previous block's softmax-denominator chain and dense groups. AV
matmuls trail their logits by 4 chunks (software pipeline) so they never
wait on the exp; the O-accumulator reuse hazard is resolved by giving
the denorm chain queue priority (behind a 2-thunk head that buys the
sums staging some slack).
"""

import numpy as np
import ml_dtypes
from contextlib import ExitStack

import concourse.tile as tile
from concourse import bacc, mybir
from concourse.bass_utils import run_bass_kernel_spmd

F32 = mybir.dt.float32
BF16 = mybir.dt.bfloat16
AF = mybir.ActivationFunctionType
ADD = mybir.AluOpType.add
MULT = mybir.AluOpType.mult

NPBF = ml_dtypes.bfloat16

B, S, D, H = 2, 2048, 1024, 16
NCORES = 8
HL = 4            # heads per core
DH = D // H       # 64
DLOC = HL * DH    # 256 local feature dims
SBK = 512         # seq block (q)
NSB = S // SBK    # 4
KCH = 128         # k chunk


def _ts(i, n):
    return slice(i * n, (i + 1) * n)


def build(causal=True, with_bq=False, with_bk=False, with_bv=False,
          x_bufs=2, pt_bufs=12, ev_bufs=5, small_bufs=4, ot_bufs=4):
    nc = bacc.Bacc(None, target_bir_lowering=False)

    xqT = nc.dram_tensor("xqT", [D, S], BF16, kind="ExternalInput")
    xkT = nc.dram_tensor("xkT", [D, S], BF16, kind="ExternalInput")
    xvT = nc.dram_tensor("xvT", [D, S], BF16, kind="ExternalInput")
    wqT = nc.dram_tensor("wqT", [D, DLOC], BF16, kind="ExternalInput")
    wkT = nc.dram_tensor("wkT", [D, DLOC], BF16, kind="ExternalInput")
    wvT = nc.dram_tensor("wvT", [D, DLOC], BF16, kind="ExternalInput")
    dnT = nc.dram_tensor("dnT", [DLOC, D], BF16, kind="ExternalInput")
    if not causal:
        mskT = nc.dram_tensor("mskT", [S, S], F32, kind="ExternalInput")
    bq = nc.dram_tensor("bq", [DLOC], BF16, kind="ExternalInput") if with_bq else None
    bk = nc.dram_tensor("bk", [DLOC], BF16, kind="ExternalInput") if with_bk else None
    bv = nc.dram_tensor("bv", [DLOC], BF16, kind="ExternalInput") if with_bv else None
    outT = nc.dram_tensor("outT", [D, S], BF16, kind="ExternalOutput")

    ones512 = nc.inline_tensor(np.ones((1, 512), NPBF), name="ones512")
    ones128p = nc.inline_tensor(np.ones((128, 1), NPBF), name="ones128p")
    ones6464 = nc.inline_tensor(np.ones((128, 64), NPBF), name="ones6464")
    # upper (inclusive) triangle: tri01[r, c] = 1 if r <= c else 0
    tri_np = np.triu(np.ones((KCH, KCH), np.float32)).astype(NPBF)
    tri_in = nc.inline_tensor(tri_np, name="tri01")

    with tile.TileContext(nc) as tc, ExitStack() as ctx:
        pers = ctx.enter_context(tc.tile_pool(name="pers", bufs=1))
        xpool = ctx.enter_context(tc.tile_pool(name="xpool", bufs=x_bufs))
        ptp = ctx.enter_context(tc.tile_pool(name="ptp", bufs=pt_bufs))
        otp = ctx.enter_context(tc.tile_pool(name="otp", bufs=ot_bufs))
        evp = ctx.enter_context(tc.tile_pool(name="evp", bufs=ev_bufs))
        smallp = ctx.enter_context(tc.tile_pool(name="smallp", bufs=small_bufs))
        if not causal:
            mskp = ctx.enter_context(tc.tile_pool(name="mskp", bufs=3))
        mmp = ctx.enter_context(tc.tile_pool(name="mmp", bufs=2, space="PSUM"))
        lp = ctx.enter_context(tc.tile_pool(name="lp", bufs=2, space="PSUM"))
        opp = ctx.enter_context(tc.tile_pool(name="opp", bufs=1, space="PSUM"))

        # ---------- persistent tiles ----------
        wsb = {}
        for wname in ("q", "k", "v"):
            wsb[wname] = pers.tile([128, 8, DLOC], BF16, tag=f"w{wname}",
                                   name=f"w_{wname}")
        dn_sb = pers.tile([128, 2, D], BF16, tag="dn")
        tri01 = pers.tile([KCH, KCH], BF16, tag="tri01")
        ones_r = pers.tile([1, 512], BF16, tag="ones_r")
        onescol = pers.tile([128, 1], BF16, tag="onescol")
        ones64 = pers.tile([128, 64], BF16, tag="ones64")

        QT_sb = pers.tile([128, 2, S], BF16, tag="QT")
        KT_sb = pers.tile([128, 2, S], BF16, tag="KT")
        V_sb = pers.tile([128, S // KCH, HL, DH + 1], BF16, tag="V")

        bsb = {}
        for name, dram in (("q", bq), ("k", bk), ("v", bv)):
            if dram is not None:
                t = pers.tile([1, DLOC], BF16, tag=f"b{name}")
                nc.sync.dma_start(out=t, in_=dram[None, :])
                bsb[name] = t

        xdram = {"q": xqT, "k": xkT, "v": xvT}
        wdram = {"q": wqT, "k": wkT, "v": wvT}
        xt_pend = {}

        def load_x(j, js):
            xt = {}
            for xname in ("q", "k", "v"):
                srcr = xdram[xname].rearrange("(c p) s -> p c s", p=128)
                t = xpool.tile([128, 8, SBK], BF16, tag=f"x{xname}",
                               name=f"x_{xname}_{j}")
                # two-way split so the first consuming matmuls can start
                # halfway through the block's transfer
                nc.sync.dma_start(out=t[:, 0:4, :], in_=srcr[:, 0:4, js])
                nc.sync.dma_start(out=t[:, 4:8, :], in_=srcr[:, 4:8, js])
                xt[xname] = t
            return xt

        js0 = _ts(0, SBK)
        for xname in ("q", "k", "v"):
            wr = wdram[xname].rearrange("(c p) m -> p c m", p=128)
            if xname == "q":
                # fine split: the first LDWEIGHTS needs only wq[kc=0]
                nc.sync.dma_start(out=wsb[xname][:, 0:1, :], in_=wr[:, 0:1, :])
                nc.sync.dma_start(out=wsb[xname][:, 1:4, :], in_=wr[:, 1:4, :])
                nc.sync.dma_start(out=wsb[xname][:, 4:8, :], in_=wr[:, 4:8, :])
            else:
                nc.sync.dma_start(out=wsb[xname], in_=wr)
            srcr = xdram[xname].rearrange("(c p) s -> p c s", p=128)
            t = xpool.tile([128, 8, SBK], BF16, tag=f"x{xname}",
                           name=f"x_{xname}_0")
            # 4-way split: the startup is DMA-bound, so let the first
            # projection matmuls start a quarter of the way in
            for q4 in range(4):
                nc.sync.dma_start(out=t[:, _ts(q4, 2), :],
                                  in_=srcr[:, _ts(q4, 2), js0])
            xt_pend[0] = xt_pend.get(0, {})
            xt_pend[0][xname] = t

        # consts after the first x block: tiny, and none is needed before
        # the first V-projection evac (~12us in)
        nc.sync.dma_start(out=ones_r, in_=ones512[:, :])
        nc.sync.dma_start(out=onescol, in_=ones128p[:, :])
        nc.sync.dma_start(out=ones64, in_=ones6464[:, :])
        # ones column of V_aug (softmax denominator trick)
        nc.vector.tensor_copy(
            V_sb[:, :, :, DH:DH + 1],
            onescol[:, None, None, :].broadcast_to([128, S // KCH, HL, 1]),
        )

        outT_r = outT.rearrange("(c p) s -> p c s", p=128)

        # round-robin copy engines for PSUM evacuation (ACT is reserved for
        # exp during attention; phase-A-only copies may use it)
        def copier(engines=("vector",), _state={}):
            k = engines
            i = _state.get(k, 0)
            _state[k] = i + 1
            eng = getattr(nc, engines[i % len(engines)])

            def cp(out, in_):
                if hasattr(eng, "tensor_copy"):
                    eng.tensor_copy(out, in_)
                else:
                    eng.copy(out=out, in_=in_)
            return type("C", (), {"tensor_copy": staticmethod(cp)})

        def phase_A_groups(j, js, xt, evac=("vector",)):
            # ---------- projections for s-block j, as 8 independent
            # matmul-group thunks so they can be interleaved into phase B ----
            def qk_group(bname, dst, mc):
                def emit():
                    ps = mmp.tile([128, 512], F32, tag="mm")
                    has_b = bname in bsb
                    for kc in range(8):
                        nc.tensor.matmul(
                            ps[:, :],
                            lhsT=wsb[bname][:, kc, _ts(mc, 128)],
                            rhs=xt[bname][:, kc, :],
                            start=(kc == 0), stop=(kc == 7 and not has_b),
                        )
                    if has_b:
                        nc.tensor.matmul(
                            ps[:, :], lhsT=bsb[bname][0:1, _ts(mc, 128)],
                            rhs=ones_r[0:1, 0:SBK], start=False, stop=True,
                        )
                    copier(evac).tensor_copy(dst[:, mc, js], ps)
                return emit

            def v_group(sc):
                def emit():
                    ps = mmp.tile([128, 512], F32, tag="mm")
                    has_b = "v" in bsb
                    for kc in range(8):
                        nc.tensor.matmul(
                            ps[:, 0:DLOC],
                            lhsT=xt["v"][:, kc, _ts(sc, 128)],
                            rhs=wsb["v"][:, kc, :],
                            start=(kc == 0), stop=(kc == 7 and not has_b),
                        )
                    if has_b:
                        nc.tensor.matmul(
                            ps[:, 0:DLOC], lhsT=ones_r[0:1, 0:128],
                            rhs=bsb["v"][0:1, :], start=False, stop=True,
                        )
                    copier(evac).tensor_copy(
                        V_sb[:, j * 4 + sc, :, 0:DH],
                        ps[:, 0:DLOC].rearrange("p (h d) -> p h d", h=HL),
                    )
                return emit

            return ([qk_group(b, d, mc) for b, d in (("q", QT_sb), ("k", KT_sb))
                     for mc in range(2)] + [v_group(sc) for sc in range(4)])

        def denorm_thunks(j, pc, O, OTs, sm):
            # softmax denominators: sums row (lane 64, staged to SBUF by the
            # caller right after the AV flush), broadcast to partitions 0:64
            # via K=1 matmul, then a single fused divide straight out of the
            # O accumulator in PSUM. Drip-fed into the NEXT attention loop so
            # the PE never blocks on the cross-engine chain.
            otpair = otp.tile([128, 512], BF16, tag=f"otp{pc}",
                              name=f"otp_{j}_{pc}")
            OTs[pc] = otpair
            st = {}

            def t_bcast():
                for i in range(2):
                    Sps = mmp.tile([128, 512], F32, tag="mm",
                                   name=f"sps_{j}_{pc}_{i}")
                    nc.tensor.matmul(
                        Sps[0:64, :], lhsT=ones64[64:65, 0:64],
                        rhs=sm[i][64:65, :],
                        start=True, stop=True, tile_position=(64, 0),
                    )
                    st[i] = Sps

            def t_rc():
                # DVE ops may read at most ONE operand from PSUM, so take the
                # reciprocal into SBUF first; the multiply then pairs the O
                # accumulator (PSUM) with rc (SBUF)
                for i in range(2):
                    rc = smallp.tile([64, 512], F32, tag="rc",
                                     name=f"rc_{j}_{pc}_{i}")
                    nc.vector.reciprocal_approx_fast(out=rc, in_=st[i][0:64, :])
                    st[i] = rc

            def t_mul0():
                nc.vector.tensor_tensor(
                    out=otpair[0:64, :], in0=O[0][0:64, :],
                    in1=st[0], op=MULT)

            def t_mul1():
                ot_tmp = smallp.tile([64, 512], BF16, tag="ott",
                                     name=f"ott_{j}_{pc}")
                nc.vector.tensor_tensor(
                    out=ot_tmp, in0=O[1][0:64, :],
                    in1=st[1], op=MULT)
                # partition shift 0:64 -> 64:128 (gpsimd can remap
                # partitions SBUF->SBUF without the DMA latency chain)
                nc.gpsimd.tensor_copy(otpair[64:128, :], ot_tmp[:, :])

            return [t_bcast, t_rc, t_mul0, t_mul1]

        def dense_thunks(j, js, OTs):
            box = {}

            def grp(mc):
                def emit():
                    if j == NSB - 1 and mc % 2:
                        # tail: the attention loops are over, so the L pool's
                        # banks are free — rotate through them too to double
                        # the dense pipeline depth
                        dps = lp.tile([128, 2, SBK], F32, tag="L",
                                      name=f"dpsl_{j}_{mc}")[:, 0, :]
                    else:
                        dps = mmp.tile([128, 512], F32, tag="mm",
                                       name=f"dps_{j}_{mc}")
                    for pc in range(2):
                        nc.tensor.matmul(
                            dps[:, :], lhsT=dn_sb[:, pc, _ts(mc, 128)],
                            rhs=OTs[pc][:, :], start=(pc == 0), stop=(pc == 1),
                        )
                    # pair consecutive mc's into one ev tile / one out-DMA
                    # (halves the 625ns-per-DMA HWDGE serialization)
                    if mc % 2 == 0:
                        box["ev"] = evp.tile([128, 2, 512], BF16, tag="ev",
                                             name=f"ev_{j}_{mc}")
                    ev = box["ev"]
                    if j == NSB - 1:
                        # tail: exps are done, ACT is free — split each evac
                        # across DVE and ACT to halve the drain
                        nc.vector.tensor_copy(ev[:, mc % 2, 0:256],
                                              dps[:, 0:256])
                        nc.scalar.copy(out=ev[:, mc % 2, 256:512],
                                       in_=dps[:, 256:512])
                    else:
                        copier().tensor_copy(ev[:, mc % 2, :], dps)
                    if mc % 2:
                        nc.sync.dma_start(
                            out=outT_r[:, mc - 1:mc + 1, js],
                            in_=ev)
                return emit
            return [grp(mc) for mc in range(8)]

        def phase_B(j, js, fill, lowq):
            # ---------- attention + dense for q-block j ----------
            # `fill`: queue of thunks (phase-A groups, previous block's
            # denorm + dense), one emitted per kc iteration right after the
            # logits matmuls, where the PE would otherwise wait on the exp.
            nkc = (j + 1) * 4 if causal else S // KCH
            OTs = [None, None]
            for pc in range(2):
                O = [
                    opp.tile([65, 512], F32, tag=f"o{i}", name=f"O_{j}_{pc}_{i}")
                    for i in range(2)
                ]
                pend = []  # software pipeline: AV trails logits by PD kc's

                def emit_av(kc, off, last, PT):
                    for i in range(2):
                        nc.tensor.matmul(
                            O[i][0:65, off:SBK],
                            lhsT=V_sb[:, kc, 2 * pc + i, :],
                            rhs=PT[:, i, off:SBK],
                            start=(kc == 0), stop=last,
                            skip_group_check=True,
                        )

                for kc in range(nkc):
                    diag = causal and kc >= 4 * j
                    off = (kc - 4 * j) * KCH if diag else 0
                    L = lp.tile([128, 2, SBK], F32, tag="L")
                    for i in range(2):
                        nc.tensor.matmul(
                            L[:, i, off:SBK],
                            lhsT=KT_sb[_ts(i, 64), pc, _ts(kc, KCH)],
                            rhs=QT_sb[_ts(i, 64), pc,
                                      j * SBK + off:(j + 1) * SBK],
                            start=True, stop=True,
                            tile_position=(64 * i, 0),
                        )
                    if fill and not (j == NSB - 1 and pc == 0
                                     and kc % 2 and kc > 4):
                        fill.pop(0)()
                    elif lowq and (j != 2 or kc % 2):
                        lowq.pop(0)()
                    if not causal:
                        mk = mskp.tile([128, SBK], F32, tag="mk")
                        nc.sync.dma_start(out=mk, in_=mskT[_ts(kc, KCH), js])
                        nc.vector.tensor_tensor(
                            out=L[:, :, :], in0=L[:, :, :],
                            in1=mk[:, None, :].broadcast_to([128, 2, SBK]),
                            op=ADD,
                        )
                    PT = ptp.tile([128, 2, SBK], BF16, tag="PT")
                    nc.scalar.activation(
                        out=PT[:, :, off:SBK], in_=L[:, :, off:SBK],
                        func=AF.Exp, scale=0.125)
                    if diag:
                        # triangle mask as 0/1 multiply (off the exp edge)
                        nc.vector.tensor_tensor(
                            out=PT[:, :, off:off + KCH],
                            in0=PT[:, :, off:off + KCH],
                            in1=tri01[:, None, :].broadcast_to([128, 2, KCH]),
                            op=MULT,
                        )
                    pend.append((kc, off, kc == nkc - 1, PT))
                    if len(pend) > 4:
                        emit_av(*pend.pop(0))
                for p in pend:
                    # absorb each flushed AV's exp-wait with leftover work
                    if j == NSB - 1 and fill:
                        fill.pop(0)()
                    elif j == NSB - 1 and lowq:
                        lowq.pop(0)()
                    emit_av(*p)
                # stage the sums rows to SBUF immediately (frees the O
                # accumulators' WAR hazard early; PE isn't involved)
                sm = []
                for i in range(2):
                    t = smallp.tile([65, 512], BF16, tag="sm",
                                    name=f"sm_{j}_{pc}_{i}")
                    if j == NSB - 1 and pc == 1:
                        # tail: DVE is busy draining dense evacs; ACT is free
                        nc.scalar.copy(out=t[64:65, :], in_=O[i][64:65, :])
                    else:
                        copier().tensor_copy(t[64:65, :], O[i][64:65, :])
                    sm.append(t)
                # denorm gets priority over queued dense/A thunks EXCEPT one
                # older thunk kept in front: it buys the sums-staging copies a
                # full iteration of slack before the broadcast matmul reads
                # them (the next round's first AV still unblocks early enough)
                head, rest = fill[:2], fill[2:]
                fill[:] = head + denorm_thunks(j, pc, O, OTs, sm) + rest
            # dense is latency-tolerant (ot_bufs=4 removes the tile-ring
            # coupling): hold it in a low-priority queue for iterations with
            # no other fill — that is mostly the bare stretches of B2/B3
            lowq.extend(dense_thunks(j, js, OTs))

        # schedule: A0, A1 up front (PE runway; ACT free, so evacs may use
        # it), then B(j) with A(j+2)'s groups and B(j-1)'s denorm + dense
        # drip-fed into the exp-paced attention loops.
        xts = {0: xt_pend.pop(0)}
        for j in range(NSB):
            if j + 1 not in xts and j + 1 < NSB:
                xts[j + 1] = load_x(j + 1, _ts(j + 1, SBK))
            if j < 2:
                for g in phase_A_groups(j, _ts(j, SBK), xts.pop(j),
                                        evac=("scalar", "vector")):
                    g()
                if j == 0:
                    # deprioritized loads: queued behind the first x-blocks;
                    # still well ahead of their first consumers
                    nc.sync.dma_start(out=tri01, in_=tri_in[:, :])
                    nc.sync.dma_start(
                        out=dn_sb,
                        in_=dnT.rearrange("(c p) n -> p c n", p=128))
        fill = []
        lowq = []
        for j in range(NSB):
            if j + 2 < NSB:
                # A2 feeds B0's fills; A3 waits until B1 (keeps B1 from
                # starving once its dense moved to the low-priority queue)
                fill.extend(phase_A_groups(j + 2, _ts(j + 2, SBK),
                                           xts.pop(j + 2)))
            phase_B(j, _ts(j, SBK), fill, lowq)
        while fill:
            fill.pop(0)()
        while lowq:
            lowq.pop(0)()

    nc.finalize()
    return nc


_CACHE = {}


def _get_nc(causal, with_bq, with_bk, with_bv):
    key = (causal, with_bq, with_bk, with_bv)
    if key not in _CACHE:
        _CACHE[key] = build(causal, with_bq, with_bk, with_bv)
    return _CACHE[key]


def _bf(a):
    return np.ascontiguousarray(a).astype(NPBF)


def _prep_in_maps(query, key_, value, mask2d, causal, wq_w, wk_w, wv_w, dense_w,
                  wq_b, wk_b, wv_b, with_bq, with_bk, with_bv):
    in_maps = []
    xT = {}
    for b in range(B):
        xT[b] = (_bf(query[b].T), _bf(key_[b].T), _bf(value[b].T))
    mskT = None if causal else np.ascontiguousarray(mask2d.T * np.float32(-8e9))
    for c in range(NCORES):
        b, g = divmod(c, 4)
        sl = _ts(g, DLOC)
        m = {
            "xqT": xT[b][0], "xkT": xT[b][1], "xvT": xT[b][2],
            "wqT": _bf(wq_w[sl].T),
            "wkT": _bf(wk_w[sl].T),
            "wvT": _bf(wv_w[sl].T),
            "dnT": _bf(dense_w[:, sl].T),
        }
        if not causal:
            m["mskT"] = mskT
        if with_bq:
            m["bq"] = _bf(wq_b[sl])
        if with_bk:
            m["bk"] = _bf(wk_b[sl])
        if with_bv:
            m["bv"] = _bf(wv_b[sl])
        in_maps.append(m)
    return in_maps


def _run(in_maps, causal, with_bq, with_bk, with_bv, **kw):
    nc = _get_nc(causal, with_bq, with_bk, with_bv)
    return run_bass_kernel_spmd(nc, in_maps, core_ids=list(range(NCORES)), **kw)


def kernel(query, key_, value, mask, wq_w, wq_b, wk_w, wk_b, wv_w, wv_b,
           dense_w, dense_b, _profile_kw=None):
    query = np.asarray(query, np.float32)
    key_ = np.asarray(key_, np.float32)
    value = np.asarray(value, np.float32)
    mask2d = np.asarray(mask, np.float32).reshape(S, S)
    wq_w = np.asarray(wq_w, np.float32)
    wk_w = np.asarray(wk_w, np.float32)
    wv_w = np.asarray(wv_w, np.float32)
    dense_w = np.asarray(dense_w, np.float32)
    wq_b = np.asarray(wq_b, np.float32)
    wk_b = np.asarray(wk_b, np.float32)
    wv_b = np.asarray(wv_b, np.float32)
    dense_b = np.asarray(dense_b, np.float32)

    causal = bool(np.array_equal(mask2d, np.triu(np.ones((S, S), np.float32), k=1)))
    with_bq = bool(np.any(wq_b))
    with_bk = bool(np.any(wk_b))
    with_bv = bool(np.any(wv_b))

    in_maps = _prep_in_maps(query, key_, value, mask2d, causal, wq_w, wk_w, wv_w,
                            dense_w, wq_b, wk_b, wv_b, with_bq, with_bk, with_bv)
    res = _run(in_maps, causal, with_bq, with_bk, with_bv, **(_profile_kw or {}))

    out = np.empty((B, S, D), np.float32)
    for b in range(B):
        acc = res.results[4 * b]["outT"].astype(np.float32).copy()
        for g in range(1, 4):
            acc += res.results[4 * b + g]["outT"]
        out[b] = acc.T + dense_b[None, :]
    if _profile_kw:
        return out, res
    return out


# revision 68
# speedup vs baseline: 1.0542x; 1.0062x over previous
"""Multi-head attention (B=2, S=2048, D=1024, H=16) on 8 TRN2 NeuronCores.

Sharding: batch x head-group. Core c handles batch c//4 and heads
[4*(c%4), 4*(c%4)+4). Each core computes its heads' Q/K/V projections
(column-parallel), causal attention, and a row-parallel partial of the
output projection. The host sums the 4 bf16 partials per batch
(all-reduce done on host during unshard) and adds dense_b.

All streaming data is bf16 (halves HBM traffic and keeps every matmul at
1 PE-cycle/row regardless of free-size); accumulation stays in fp32
PSUM. On-core dataflow (transposed, feature-major):
  QT = WqT.T @ XqT   [256, 2048]   (bf16 matmuls, K=1024 in 8 chunks)
  KT, V likewise (V in natural [S, 256] layout, + ones column for row sums)
  per head pair, per 512-wide q-block, per 128-wide k-chunk:
    logitsT [sk=128, sq] = KT_chunk.T @ QT_block   (2 heads row-packed
      at tile_position (0,0)/(64,0), shared 2-bank PSUM tile); for
      diagonal chunks only columns [off:512] are computed (causal clip)
    PT = exp(0.125 * logitsT) -> bf16     (ScalarE, PSUM->SBUF)
    diagonal chunks: multiply PT[off:off+128] by 0/1 upper-tri (DVE)
    OT[65, off:512] += V_aug.T @ PT    (V_aug = [V | ones] -> rows 0:64 =
                                        O^T, row 64 = softmax denominator)
  per head: sums broadcast to rows 0:64 via K=1 matmul; rc = 1/sums (DVE
    reciprocal into SBUF - DVE ops may read only one PSUM operand);
    OT_norm = OT * rc (head-odd shifted to partitions 64:128 by GPSIMD);
    dense partial outT[., q-block] += denseT.T @ OT_norm, evacuated in
    mc-pairs sharing one HBM DMA
Causality: fully-masked k-chunks are skipped and diagonal chunks are
column-clipped (~2x less work). A generic path (any mask) adds
maskT * -8e9 to every chunk instead.

Schedule: the attention loop is exp-paced (ScalarE ~1040ns per k-chunk
vs ~850ns of PE work), so everything else is drip-fed into it as "fill"
thunks consumed one per iteration: the projections for block j+2, and
the previous block's softmax-denominator chain and dense groups. AV
matmuls trail their logits by 4 chunks (software pipeline) so they never
wait on the exp; the O-accumulator reuse hazard is resolved by giving
the denorm chain queue priority (behind a 2-thunk head that buys the
sums staging some slack).
"""

import numpy as np
import ml_dtypes
from contextlib import ExitStack

import concourse.tile as tile
from concourse import bacc, mybir
from concourse.bass_utils import run_bass_kernel_spmd

F32 = mybir.dt.float32
BF16 = mybir.dt.bfloat16
AF = mybir.ActivationFunctionType
ADD = mybir.AluOpType.add
MULT = mybir.AluOpType.mult

NPBF = ml_dtypes.bfloat16

B, S, D, H = 2, 2048, 1024, 16
NCORES = 8
HL = 4            # heads per core
DH = D // H       # 64
DLOC = HL * DH    # 256 local feature dims
SBK = 512         # seq block (q)
NSB = S // SBK    # 4
KCH = 128         # k chunk


def _ts(i, n):
    return slice(i * n, (i + 1) * n)


def build(causal=True, with_bq=False, with_bk=False, with_bv=False,
          x_bufs=2, pt_bufs=12, ev_bufs=5, small_bufs=4, ot_bufs=4):
    nc = bacc.Bacc(None, target_bir_lowering=False)

    xqT = nc.dram_tensor("xqT", [D, S], BF16, kind="ExternalInput")
    xkT = nc.dram_tensor("xkT", [D, S], BF16, kind="ExternalInput")
    xvT = nc.dram_tensor("xvT", [D, S], BF16, kind="ExternalInput")
    wqT = nc.dram_tensor("wqT", [D, DLOC], BF16, kind="ExternalInput")
    wkT = nc.dram_tensor("wkT", [D, DLOC], BF16, kind="ExternalInput")
    wvT = nc.dram_tensor("wvT", [D, DLOC], BF16, kind="ExternalInput")
    dnT = nc.dram_tensor("dnT", [DLOC, D], BF16, kind="ExternalInput")
    if not causal:
        mskT = nc.dram_tensor("mskT", [S, S], F32, kind="ExternalInput")
    bq = nc.dram_tensor("bq", [DLOC], BF16, kind="ExternalInput") if with_bq else None
    bk = nc.dram_tensor("bk", [DLOC], BF16, kind="ExternalInput") if with_bk else None
    bv = nc.dram_tensor("bv", [DLOC], BF16, kind="ExternalInput") if with_bv else None
    outT = nc.dram_tensor("outT", [D, S], BF16, kind="ExternalOutput")

    ones512 = nc.inline_tensor(np.ones((1, 512), NPBF), name="ones512")
    ones128p = nc.inline_tensor(np.ones((128, 1), NPBF), name="ones128p")
    ones6464 = nc.inline_tensor(np.ones((128, 64), NPBF), name="ones6464")
    # upper (inclusive) triangle: tri01[r, c] = 1 if r <= c else 0
    tri_np = np.triu(np.ones((KCH, KCH), np.float32)).astype(NPBF)
    tri_in = nc.inline_tensor(tri_np, name="tri01")

    with tile.TileContext(nc) as tc, ExitStack() as ctx:
        pers = ctx.enter_context(tc.tile_pool(name="pers", bufs=1))
        xpool = ctx.enter_context(tc.tile_pool(name="xpool", bufs=x_bufs))
        ptp = ctx.enter_context(tc.tile_pool(name="ptp", bufs=pt_bufs))
        otp = ctx.enter_context(tc.tile_pool(name="otp", bufs=ot_bufs))
        evp = ctx.enter_context(tc.tile_pool(name="evp", bufs=ev_bufs))
        smallp = ctx.enter_context(tc.tile_pool(name="smallp", bufs=small_bufs))
        if not causal:
            mskp = ctx.enter_context(tc.tile_pool(name="mskp", bufs=3))
        mmp = ctx.enter_context(tc.tile_pool(name="mmp", bufs=2, space="PSUM"))
        lp = ctx.enter_context(tc.tile_pool(name="lp", bufs=2, space="PSUM"))
        opp = ctx.enter_context(tc.tile_pool(name="opp", bufs=1, space="PSUM"))

        # ---------- persistent tiles ----------
        wsb = {}
        for wname in ("q", "k", "v"):
            wsb[wname] = pers.tile([128, 8, DLOC], BF16, tag=f"w{wname}",
                                   name=f"w_{wname}")
        dn_sb = pers.tile([128, 2, D], BF16, tag="dn")
        tri01 = pers.tile([KCH, KCH], BF16, tag="tri01")
        ones_r = pers.tile([1, 512], BF16, tag="ones_r")
        onescol = pers.tile([128, 1], BF16, tag="onescol")
        ones64 = pers.tile([128, 64], BF16, tag="ones64")

        QT_sb = pers.tile([128, 2, S], BF16, tag="QT")
        KT_sb = pers.tile([128, 2, S], BF16, tag="KT")
        V_sb = pers.tile([128, S // KCH, HL, DH + 1], BF16, tag="V")

        bsb = {}
        for name, dram in (("q", bq), ("k", bk), ("v", bv)):
            if dram is not None:
                t = pers.tile([1, DLOC], BF16, tag=f"b{name}")
                nc.sync.dma_start(out=t, in_=dram[None, :])
                bsb[name] = t

        xdram = {"q": xqT, "k": xkT, "v": xvT}
        wdram = {"q": wqT, "k": wkT, "v": wvT}
        xt_pend = {}

        def load_x(j, js):
            xt = {}
            for xname in ("q", "k", "v"):
                srcr = xdram[xname].rearrange("(c p) s -> p c s", p=128)
                t = xpool.tile([128, 8, SBK], BF16, tag=f"x{xname}",
                               name=f"x_{xname}_{j}")
                # two-way split so the first consuming matmuls can start
                # halfway through the block's transfer
                nc.sync.dma_start(out=t[:, 0:4, :], in_=srcr[:, 0:4, js])
                nc.sync.dma_start(out=t[:, 4:8, :], in_=srcr[:, 4:8, js])
                xt[xname] = t
            return xt

        js0 = _ts(0, SBK)
        for xname in ("q", "k", "v"):
            wr = wdram[xname].rearrange("(c p) m -> p c m", p=128)
            if xname == "q":
                # fine split: the first LDWEIGHTS needs only wq[kc=0]
                nc.sync.dma_start(out=wsb[xname][:, 0:1, :], in_=wr[:, 0:1, :])
                nc.sync.dma_start(out=wsb[xname][:, 1:4, :], in_=wr[:, 1:4, :])
                nc.sync.dma_start(out=wsb[xname][:, 4:8, :], in_=wr[:, 4:8, :])
            else:
                nc.sync.dma_start(out=wsb[xname], in_=wr)
            srcr = xdram[xname].rearrange("(c p) s -> p c s", p=128)
            t = xpool.tile([128, 8, SBK], BF16, tag=f"x{xname}",
                           name=f"x_{xname}_0")
            # 4-way split: the startup is DMA-bound, so let the first
            # projection matmuls start a quarter of the way in
            for q4 in range(4):
                nc.sync.dma_start(out=t[:, _ts(q4, 2), :],
                                  in_=srcr[:, _ts(q4, 2), js0])
            xt_pend[0] = xt_pend.get(0, {})
            xt_pend[0][xname] = t

        # consts after the first x block: tiny, and none is needed before
        # the first V-projection evac (~12us in)
        nc.sync.dma_start(out=ones_r, in_=ones512[:, :])
        nc.sync.dma_start(out=onescol, in_=ones128p[:, :])
        nc.sync.dma_start(out=ones64, in_=ones6464[:, :])
        # ones column of V_aug (softmax denominator trick)
        nc.vector.tensor_copy(
            V_sb[:, :, :, DH:DH + 1],
            onescol[:, None, None, :].broadcast_to([128, S // KCH, HL, 1]),
        )

        outT_r = outT.rearrange("(c p) s -> p c s", p=128)

        # round-robin copy engines for PSUM evacuation (ACT is reserved for
        # exp during attention; phase-A-only copies may use it)
        def copier(engines=("vector",), _state={}):
            k = engines
            i = _state.get(k, 0)
            _state[k] = i + 1
            eng = getattr(nc, engines[i % len(engines)])

            def cp(out, in_):
                if hasattr(eng, "tensor_copy"):
                    eng.tensor_copy(out, in_)
                else:
                    eng.copy(out=out, in_=in_)
            return type("C", (), {"tensor_copy": staticmethod(cp)})

        def phase_A_groups(j, js, xt, evac=("vector",)):
            # ---------- projections for s-block j, as 8 independent
            # matmul-group thunks so they can be interleaved into phase B ----
            def qk_group(bname, dst, mc):
                def emit():
                    ps = mmp.tile([128, 512], F32, tag="mm")
                    has_b = bname in bsb
                    for kc in range(8):
                        nc.tensor.matmul(
                            ps[:, :],
                            lhsT=wsb[bname][:, kc, _ts(mc, 128)],
                            rhs=xt[bname][:, kc, :],
                            start=(kc == 0), stop=(kc == 7 and not has_b),
                        )
                    if has_b:
                        nc.tensor.matmul(
                            ps[:, :], lhsT=bsb[bname][0:1, _ts(mc, 128)],
                            rhs=ones_r[0:1, 0:SBK], start=False, stop=True,
                        )
                    copier(evac).tensor_copy(dst[:, mc, js], ps)
                return emit

            def v_group(sc):
                def emit():
                    ps = mmp.tile([128, 512], F32, tag="mm")
                    has_b = "v" in bsb
                    for kc in range(8):
                        nc.tensor.matmul(
                            ps[:, 0:DLOC],
                            lhsT=xt["v"][:, kc, _ts(sc, 128)],
                            rhs=wsb["v"][:, kc, :],
                            start=(kc == 0), stop=(kc == 7 and not has_b),
                        )
                    if has_b:
                        nc.tensor.matmul(
                            ps[:, 0:DLOC], lhsT=ones_r[0:1, 0:128],
                            rhs=bsb["v"][0:1, :], start=False, stop=True,
                        )
                    copier(evac).tensor_copy(
                        V_sb[:, j * 4 + sc, :, 0:DH],
                        ps[:, 0:DLOC].rearrange("p (h d) -> p h d", h=HL),
                    )
                return emit

            return ([qk_group(b, d, mc) for b, d in (("q", QT_sb), ("k", KT_sb))
                     for mc in range(2)] + [v_group(sc) for sc in range(4)])

        def denorm_thunks(j, pc, O, OTs, sm):
            # softmax denominators: sums row (lane 64, staged to SBUF by the
            # caller right after the AV flush), broadcast to partitions 0:64
            # via K=1 matmul, then a single fused divide straight out of the
            # O accumulator in PSUM. Drip-fed into the NEXT attention loop so
            # the PE never blocks on the cross-engine chain.
            otpair = otp.tile([128, 512], BF16, tag=f"otp{pc}",
                              name=f"otp_{j}_{pc}")
            OTs[pc] = otpair
            st = {}

            def t_bcast():
                for i in range(2):
                    Sps = mmp.tile([128, 512], F32, tag="mm",
                                   name=f"sps_{j}_{pc}_{i}")
                    nc.tensor.matmul(
                        Sps[0:64, :], lhsT=ones64[64:65, 0:64],
                        rhs=sm[i][64:65, :],
                        start=True, stop=True, tile_position=(64, 0),
                    )
                    st[i] = Sps

            def t_rc():
                # DVE ops may read at most ONE operand from PSUM, so take the
                # reciprocal into SBUF first; the multiply then pairs the O
                # accumulator (PSUM) with rc (SBUF)
                for i in range(2):
                    rc = smallp.tile([64, 512], F32, tag="rc",
                                     name=f"rc_{j}_{pc}_{i}")
                    nc.vector.reciprocal_approx_fast(out=rc, in_=st[i][0:64, :])
                    st[i] = rc

            def t_mul0():
                nc.vector.tensor_tensor(
                    out=otpair[0:64, :], in0=O[0][0:64, :],
                    in1=st[0], op=MULT)

            def t_mul1():
                ot_tmp = smallp.tile([64, 512], BF16, tag="ott",
                                     name=f"ott_{j}_{pc}")
                nc.vector.tensor_tensor(
                    out=ot_tmp, in0=O[1][0:64, :],
                    in1=st[1], op=MULT)
                # partition shift 0:64 -> 64:128 (gpsimd can remap
                # partitions SBUF->SBUF without the DMA latency chain)
                nc.gpsimd.tensor_copy(otpair[64:128, :], ot_tmp[:, :])

            return [t_bcast, t_rc, t_mul0, t_mul1]

        def dense_thunks(j, js, OTs):
            box = {}

            def grp(mc):
                def emit():
                    if j == NSB - 1 and mc % 2:
                        # tail: the attention loops are over, so the L pool's
                        # banks are free — rotate through them too to double
                        # the dense pipeline depth
                        dps = lp.tile([128, 2, SBK], F32, tag="L",
                                      name=f"dpsl_{j}_{mc}")[:, 0, :]
                    else:
                        dps = mmp.tile([128, 512], F32, tag="mm",
                                       name=f"dps_{j}_{mc}")
                    for pc in range(2):
                        nc.tensor.matmul(
                            dps[:, :], lhsT=dn_sb[:, pc, _ts(mc, 128)],
                            rhs=OTs[pc][:, :], start=(pc == 0), stop=(pc == 1),
                        )
                    # pair consecutive mc's into one ev tile / one out-DMA
                    # (halves the 625ns-per-DMA HWDGE serialization)
                    if mc % 2 == 0:
                        box["ev"] = evp.tile([128, 2, 512], BF16, tag="ev",
                                             name=f"ev_{j}_{mc}")
                    ev = box["ev"]
                    if j == NSB - 1:
                        # tail: exps are done, ACT is free — alternate whole
                        # evacs across DVE and ACT so both queues drain the
                        # eight copies in parallel
                        if mc % 2:
                            nc.scalar.copy(out=ev[:, 1, :], in_=dps)
                        else:
                            nc.vector.tensor_copy(ev[:, 0, :], dps)
                    else:
                        copier().tensor_copy(ev[:, mc % 2, :], dps)
                    if mc % 2:
                        nc.sync.dma_start(
                            out=outT_r[:, mc - 1:mc + 1, js],
                            in_=ev)
                return emit
            return [grp(mc) for mc in range(8)]

        def phase_B(j, js, fill, lowq):
            # ---------- attention + dense for q-block j ----------
            # `fill`: queue of thunks (phase-A groups, previous block's
            # denorm + dense), one emitted per kc iteration right after the
            # logits matmuls, where the PE would otherwise wait on the exp.
            nkc = (j + 1) * 4 if causal else S // KCH
            OTs = [None, None]
            for pc in range(2):
                O = [
                    opp.tile([65, 512], F32, tag=f"o{i}", name=f"O_{j}_{pc}_{i}")
                    for i in range(2)
                ]
                pend = []  # software pipeline: AV trails logits by PD kc's

                def emit_av(kc, off, last, PT):
                    for i in range(2):
                        nc.tensor.matmul(
                            O[i][0:65, off:SBK],
                            lhsT=V_sb[:, kc, 2 * pc + i, :],
                            rhs=PT[:, i, off:SBK],
                            start=(kc == 0), stop=last,
                            skip_group_check=True,
                        )

                for kc in range(nkc):
                    diag = causal and kc >= 4 * j
                    off = (kc - 4 * j) * KCH if diag else 0
                    L = lp.tile([128, 2, SBK], F32, tag="L")
                    for i in range(2):
                        nc.tensor.matmul(
                            L[:, i, off:SBK],
                            lhsT=KT_sb[_ts(i, 64), pc, _ts(kc, KCH)],
                            rhs=QT_sb[_ts(i, 64), pc,
                                      j * SBK + off:(j + 1) * SBK],
                            start=True, stop=True,
                            tile_position=(64 * i, 0),
                        )
                    if fill and not (j == NSB - 1 and pc == 0
                                     and kc % 2 and kc > 4):
                        fill.pop(0)()
                    elif lowq and (j != 2 or kc % 2):
                        lowq.pop(0)()
                    if not causal:
                        mk = mskp.tile([128, SBK], F32, tag="mk")
                        nc.sync.dma_start(out=mk, in_=mskT[_ts(kc, KCH), js])
                        nc.vector.tensor_tensor(
                            out=L[:, :, :], in0=L[:, :, :],
                            in1=mk[:, None, :].broadcast_to([128, 2, SBK]),
                            op=ADD,
                        )
                    PT = ptp.tile([128, 2, SBK], BF16, tag="PT")
                    nc.scalar.activation(
                        out=PT[:, :, off:SBK], in_=L[:, :, off:SBK],
                        func=AF.Exp, scale=0.125)
                    if diag:
                        # triangle mask as 0/1 multiply (off the exp edge)
                        nc.vector.tensor_tensor(
                            out=PT[:, :, off:off + KCH],
                            in0=PT[:, :, off:off + KCH],
                            in1=tri01[:, None, :].broadcast_to([128, 2, KCH]),
                            op=MULT,
                        )
                    pend.append((kc, off, kc == nkc - 1, PT))
                    if len(pend) > 4:
                        emit_av(*pend.pop(0))
                for p in pend:
                    # absorb each flushed AV's exp-wait with leftover work
                    if j == NSB - 1 and fill:
                        fill.pop(0)()
                    elif j == NSB - 1 and lowq:
                        lowq.pop(0)()
                    emit_av(*p)
                # stage the sums rows to SBUF immediately (frees the O
                # accumulators' WAR hazard early; PE isn't involved)
                sm = []
                for i in range(2):
                    t = smallp.tile([65, 512], BF16, tag="sm",
                                    name=f"sm_{j}_{pc}_{i}")
                    if j == NSB - 1 and pc == 1 and i == 1:
                        # tail: ACT is still draining the last exps — put one
                        # of the two sums copies on DVE so they land together
                        nc.scalar.copy(out=t[64:65, :], in_=O[i][64:65, :])
                    else:
                        copier().tensor_copy(t[64:65, :], O[i][64:65, :])
                    sm.append(t)
                # denorm gets priority over queued dense/A thunks EXCEPT one
                # older thunk kept in front: it buys the sums-staging copies a
                # full iteration of slack before the broadcast matmul reads
                # them (the next round's first AV still unblocks early enough)
                head, rest = fill[:2], fill[2:]
                fill[:] = head + denorm_thunks(j, pc, O, OTs, sm) + rest
            # dense is latency-tolerant (ot_bufs=4 removes the tile-ring
            # coupling): hold it in a low-priority queue for iterations with
            # no other fill — that is mostly the bare stretches of B2/B3
            lowq.extend(dense_thunks(j, js, OTs))

        # schedule: A0, A1 up front (PE runway; ACT free, so evacs may use
        # it), then B(j) with A(j+2)'s groups and B(j-1)'s denorm + dense
        # drip-fed into the exp-paced attention loops.
        xts = {0: xt_pend.pop(0)}
        for j in range(NSB):
            if j + 1 not in xts and j + 1 < NSB:
                xts[j + 1] = load_x(j + 1, _ts(j + 1, SBK))
            if j < 2:
                for g in phase_A_groups(j, _ts(j, SBK), xts.pop(j),
                                        evac=("scalar", "vector")):
                    g()
                if j == 0:
                    # deprioritized loads: queued behind the first x-blocks;
                    # still well ahead of their first consumers
                    nc.sync.dma_start(out=tri01, in_=tri_in[:, :])
                    nc.sync.dma_start(
                        out=dn_sb,
                        in_=dnT.rearrange("(c p) n -> p c n", p=128))
        fill = []
        lowq = []
        for j in range(NSB):
            if j + 2 < NSB:
                # A2 feeds B0's fills; A3 waits until B1 (keeps B1 from
                # starving once its dense moved to the low-priority queue)
                fill.extend(phase_A_groups(j + 2, _ts(j + 2, SBK),
                                           xts.pop(j + 2)))
            phase_B(j, _ts(j, SBK), fill, lowq)
        while fill:
            fill.pop(0)()
        while lowq:
            lowq.pop(0)()

    nc.finalize()
    return nc


_CACHE = {}


def _get_nc(causal, with_bq, with_bk, with_bv):
    key = (causal, with_bq, with_bk, with_bv)
    if key not in _CACHE:
        _CACHE[key] = build(causal, with_bq, with_bk, with_bv)
    return _CACHE[key]


def _bf(a):
    return np.ascontiguousarray(a).astype(NPBF)


def _prep_in_maps(query, key_, value, mask2d, causal, wq_w, wk_w, wv_w, dense_w,
                  wq_b, wk_b, wv_b, with_bq, with_bk, with_bv):
    in_maps = []
    xT = {}
    for b in range(B):
        xT[b] = (_bf(query[b].T), _bf(key_[b].T), _bf(value[b].T))
    mskT = None if causal else np.ascontiguousarray(mask2d.T * np.float32(-8e9))
    for c in range(NCORES):
        b, g = divmod(c, 4)
        sl = _ts(g, DLOC)
        m = {
            "xqT": xT[b][0], "xkT": xT[b][1], "xvT": xT[b][2],
            "wqT": _bf(wq_w[sl].T),
            "wkT": _bf(wk_w[sl].T),
            "wvT": _bf(wv_w[sl].T),
            "dnT": _bf(dense_w[:, sl].T),
        }
        if not causal:
            m["mskT"] = mskT
        if with_bq:
            m["bq"] = _bf(wq_b[sl])
        if with_bk:
            m["bk"] = _bf(wk_b[sl])
        if with_bv:
            m["bv"] = _bf(wv_b[sl])
        in_maps.append(m)
    return in_maps


def _run(in_maps, causal, with_bq, with_bk, with_bv, **kw):
    nc = _get_nc(causal, with_bq, with_bk, with_bv)
    return run_bass_kernel_spmd(nc, in_maps, core_ids=list(range(NCORES)), **kw)


def kernel(query, key_, value, mask, wq_w, wq_b, wk_w, wk_b, wv_w, wv_b,
           dense_w, dense_b, _profile_kw=None):
    query = np.asarray(query, np.float32)
    key_ = np.asarray(key_, np.float32)
    value = np.asarray(value, np.float32)
    mask2d = np.asarray(mask, np.float32).reshape(S, S)
    wq_w = np.asarray(wq_w, np.float32)
    wk_w = np.asarray(wk_w, np.float32)
    wv_w = np.asarray(wv_w, np.float32)
    dense_w = np.asarray(dense_w, np.float32)
    wq_b = np.asarray(wq_b, np.float32)
    wk_b = np.asarray(wk_b, np.float32)
    wv_b = np.asarray(wv_b, np.float32)
    dense_b = np.asarray(dense_b, np.float32)

    causal = bool(np.array_equal(mask2d, np.triu(np.ones((S, S), np.float32), k=1)))
    with_bq = bool(np.any(wq_b))
    with_bk = bool(np.any(wk_b))
    with_bv = bool(np.any(wv_b))

    in_maps = _prep_in_maps(query, key_, value, mask2d, causal, wq_w, wk_w, wv_w,
                            dense_w, wq_b, wk_b, wv_b, with_bq, with_bk, with_bv)
    res = _run(in_maps, causal, with_bq, with_bk, with_bv, **(_profile_kw or {}))

    out = np.empty((B, S, D), np.float32)
    for b in range(B):
        acc = res.results[4 * b]["outT"].astype(np.float32).copy()
        for g in range(1, 4):
            acc += res.results[4 * b + g]["outT"]
        out[b] = acc.T + dense_b[None, :]
    if _profile_kw:
        return out, res
    return out


# revision 76
# speedup vs baseline: 1.0599x; 1.0053x over previous
"""Multi-head attention (B=2, S=2048, D=1024, H=16) on 8 TRN2 NeuronCores.

Sharding: batch x head-group. Core c handles batch c//4 and heads
[4*(c%4), 4*(c%4)+4). Each core computes its heads' Q/K/V projections
(column-parallel), causal attention, and a row-parallel partial of the
output projection. The host sums the 4 bf16 partials per batch
(all-reduce done on host during unshard) and adds dense_b.

All streaming data is bf16 (halves HBM traffic and keeps every matmul at
1 PE-cycle/row regardless of free-size); accumulation stays in fp32
PSUM. On-core dataflow (transposed, feature-major):
  QT = WqT.T @ XqT   [256, 2048]   (bf16 matmuls, K=1024 in 8 chunks)
  KT, V likewise (V in natural [S, 256] layout, + ones column for row sums)
  per head pair, per 512-wide q-block, per 128-wide k-chunk:
    logitsT [sk=128, sq] = KT_chunk.T @ QT_block   (2 heads row-packed
      at tile_position (0,0)/(64,0), shared 2-bank PSUM tile); for
      diagonal chunks only columns [off:512] are computed (causal clip)
    PT = exp(0.125 * logitsT) -> bf16     (ScalarE, PSUM->SBUF)
    diagonal chunks: multiply PT[off:off+128] by 0/1 upper-tri (DVE)
    OT[65, off:512] += V_aug.T @ PT    (V_aug = [V | ones] -> rows 0:64 =
                                        O^T, row 64 = softmax denominator)
  per head: sums broadcast to rows 0:64 via K=1 matmul; rc = 1/sums (DVE
    reciprocal into SBUF - DVE ops may read only one PSUM operand);
    OT_norm = OT * rc (head-odd shifted to partitions 64:128 by GPSIMD);
    dense partial outT[., q-block] += denseT.T @ OT_norm, evacuated in
    mc-pairs sharing one HBM DMA
Causality: fully-masked k-chunks are skipped and diagonal chunks are
column-clipped (~2x less work). A generic path (any mask) adds
maskT * -8e9 to every chunk instead.

Schedule: the attention loop is exp-paced (ScalarE ~1040ns per k-chunk
vs ~850ns of PE work), so everything else is drip-fed into it as "fill"
thunks consumed one per iteration: the projections for block j+2, and
the previous block's softmax-denominator chain and dense groups. AV
matmuls trail their logits by 4 chunks (software pipeline) so they never
wait on the exp; the O-accumulator reuse hazard is resolved by giving
the denorm chain queue priority (behind a 2-thunk head that buys the
sums staging some slack).
"""

import numpy as np
import ml_dtypes
from contextlib import ExitStack

import concourse.tile as tile
from concourse import bacc, mybir
from concourse.bass_utils import run_bass_kernel_spmd

F32 = mybir.dt.float32
BF16 = mybir.dt.bfloat16
AF = mybir.ActivationFunctionType
ADD = mybir.AluOpType.add
MULT = mybir.AluOpType.mult

NPBF = ml_dtypes.bfloat16

B, S, D, H = 2, 2048, 1024, 16
NCORES = 8
HL = 4            # heads per core
DH = D // H       # 64
DLOC = HL * DH    # 256 local feature dims
SBK = 512         # seq block (q)
NSB = S // SBK    # 4
KCH = 128         # k chunk


def _ts(i, n):
    return slice(i * n, (i + 1) * n)


def build(causal=True, with_bq=False, with_bk=False, with_bv=False,
          x_bufs=2, pt_bufs=12, ev_bufs=5, small_bufs=4, ot_bufs=4):
    nc = bacc.Bacc(None, target_bir_lowering=False)

    xqT = nc.dram_tensor("xqT", [D, S], BF16, kind="ExternalInput")
    xkT = nc.dram_tensor("xkT", [D, S], BF16, kind="ExternalInput")
    xvT = nc.dram_tensor("xvT", [D, S], BF16, kind="ExternalInput")
    wqT = nc.dram_tensor("wqT", [D, DLOC], BF16, kind="ExternalInput")
    wkT = nc.dram_tensor("wkT", [D, DLOC], BF16, kind="ExternalInput")
    wvT = nc.dram_tensor("wvT", [D, DLOC], BF16, kind="ExternalInput")
    dnT = nc.dram_tensor("dnT", [DLOC, D], BF16, kind="ExternalInput")
    if not causal:
        mskT = nc.dram_tensor("mskT", [S, S], F32, kind="ExternalInput")
    bq = nc.dram_tensor("bq", [DLOC], BF16, kind="ExternalInput") if with_bq else None
    bk = nc.dram_tensor("bk", [DLOC], BF16, kind="ExternalInput") if with_bk else None
    bv = nc.dram_tensor("bv", [DLOC], BF16, kind="ExternalInput") if with_bv else None
    outT = nc.dram_tensor("outT", [D, S], BF16, kind="ExternalOutput")

    ones512 = nc.inline_tensor(np.ones((1, 512), NPBF), name="ones512")
    ones128p = nc.inline_tensor(np.ones((128, 1), NPBF), name="ones128p")
    ones6464 = nc.inline_tensor(np.ones((128, 64), NPBF), name="ones6464")
    # upper (inclusive) triangle: tri01[r, c] = 1 if r <= c else 0
    tri_np = np.triu(np.ones((KCH, KCH), np.float32)).astype(NPBF)
    tri_in = nc.inline_tensor(tri_np, name="tri01")

    with tile.TileContext(nc) as tc, ExitStack() as ctx:
        pers = ctx.enter_context(tc.tile_pool(name="pers", bufs=1))
        xpool = ctx.enter_context(tc.tile_pool(name="xpool", bufs=x_bufs))
        ptp = ctx.enter_context(tc.tile_pool(name="ptp", bufs=pt_bufs))
        otp = ctx.enter_context(tc.tile_pool(name="otp", bufs=ot_bufs))
        evp = ctx.enter_context(tc.tile_pool(name="evp", bufs=ev_bufs))
        smallp = ctx.enter_context(tc.tile_pool(name="smallp", bufs=small_bufs))
        if not causal:
            mskp = ctx.enter_context(tc.tile_pool(name="mskp", bufs=3))
        mmp = ctx.enter_context(tc.tile_pool(name="mmp", bufs=2, space="PSUM"))
        lp = ctx.enter_context(tc.tile_pool(name="lp", bufs=2, space="PSUM"))
        opp = ctx.enter_context(tc.tile_pool(name="opp", bufs=1, space="PSUM"))

        # ---------- persistent tiles ----------
        wsb = {}
        for wname in ("q", "k", "v"):
            wsb[wname] = pers.tile([128, 8, DLOC], BF16, tag=f"w{wname}",
                                   name=f"w_{wname}")
        dn_sb = pers.tile([128, 2, D], BF16, tag="dn")
        tri01 = pers.tile([KCH, KCH], BF16, tag="tri01")
        ones_r = pers.tile([1, 512], BF16, tag="ones_r")
        onescol = pers.tile([128, 1], BF16, tag="onescol")
        ones64 = pers.tile([128, 64], BF16, tag="ones64")

        QT_sb = pers.tile([128, 2, S], BF16, tag="QT")
        KT_sb = pers.tile([128, 2, S], BF16, tag="KT")
        V_sb = pers.tile([128, S // KCH, HL, DH + 1], BF16, tag="V")

        bsb = {}
        for name, dram in (("q", bq), ("k", bk), ("v", bv)):
            if dram is not None:
                t = pers.tile([1, DLOC], BF16, tag=f"b{name}")
                nc.sync.dma_start(out=t, in_=dram[None, :])
                bsb[name] = t

        xdram = {"q": xqT, "k": xkT, "v": xvT}
        wdram = {"q": wqT, "k": wkT, "v": wvT}
        xt_pend = {}

        def load_x(j, js):
            xt = {}
            for xname in ("q", "k", "v"):
                srcr = xdram[xname].rearrange("(c p) s -> p c s", p=128)
                t = xpool.tile([128, 8, SBK], BF16, tag=f"x{xname}",
                               name=f"x_{xname}_{j}")
                # two-way split so the first consuming matmuls can start
                # halfway through the block's transfer
                nc.sync.dma_start(out=t[:, 0:4, :], in_=srcr[:, 0:4, js])
                nc.sync.dma_start(out=t[:, 4:8, :], in_=srcr[:, 4:8, js])
                xt[xname] = t
            return xt

        js0 = _ts(0, SBK)
        for xname in ("q", "k", "v"):
            wr = wdram[xname].rearrange("(c p) m -> p c m", p=128)
            if xname == "q":
                # fine split: the first LDWEIGHTS needs only wq[kc=0]
                nc.sync.dma_start(out=wsb[xname][:, 0:1, :], in_=wr[:, 0:1, :])
                nc.sync.dma_start(out=wsb[xname][:, 1:4, :], in_=wr[:, 1:4, :])
                nc.sync.dma_start(out=wsb[xname][:, 4:8, :], in_=wr[:, 4:8, :])
            else:
                nc.sync.dma_start(out=wsb[xname], in_=wr)
            srcr = xdram[xname].rearrange("(c p) s -> p c s", p=128)
            t = xpool.tile([128, 8, SBK], BF16, tag=f"x{xname}",
                           name=f"x_{xname}_0")
            # 4-way split: the startup is DMA-bound, so let the first
            # projection matmuls start a quarter of the way in
            for q4 in range(4):
                nc.sync.dma_start(out=t[:, _ts(q4, 2), :],
                                  in_=srcr[:, _ts(q4, 2), js0])
            xt_pend[0] = xt_pend.get(0, {})
            xt_pend[0][xname] = t

        # consts after the first x block: tiny, and none is needed before
        # the first V-projection evac (~12us in)
        nc.sync.dma_start(out=ones_r, in_=ones512[:, :])
        nc.sync.dma_start(out=onescol, in_=ones128p[:, :])
        nc.sync.dma_start(out=ones64, in_=ones6464[:, :])
        # ones column of V_aug (softmax denominator trick)
        nc.vector.tensor_copy(
            V_sb[:, :, :, DH:DH + 1],
            onescol[:, None, None, :].broadcast_to([128, S // KCH, HL, 1]),
        )

        outT_r = outT.rearrange("(c p) s -> p c s", p=128)

        # round-robin copy engines for PSUM evacuation (ACT is reserved for
        # exp during attention; phase-A-only copies may use it)
        def copier(engines=("vector",), _state={}):
            k = engines
            i = _state.get(k, 0)
            _state[k] = i + 1
            eng = getattr(nc, engines[i % len(engines)])

            def cp(out, in_):
                if hasattr(eng, "tensor_copy"):
                    eng.tensor_copy(out, in_)
                else:
                    eng.copy(out=out, in_=in_)
            return type("C", (), {"tensor_copy": staticmethod(cp)})

        def phase_A_groups(j, js, xt, evac=("vector",)):
            # ---------- projections for s-block j, as 8 independent
            # matmul-group thunks so they can be interleaved into phase B ----
            def qk_group(bname, dst, mc):
                def emit():
                    ps = mmp.tile([128, 512], F32, tag="mm")
                    has_b = bname in bsb
                    for kc in range(8):
                        nc.tensor.matmul(
                            ps[:, :],
                            lhsT=wsb[bname][:, kc, _ts(mc, 128)],
                            rhs=xt[bname][:, kc, :],
                            start=(kc == 0), stop=(kc == 7 and not has_b),
                        )
                    if has_b:
                        nc.tensor.matmul(
                            ps[:, :], lhsT=bsb[bname][0:1, _ts(mc, 128)],
                            rhs=ones_r[0:1, 0:SBK], start=False, stop=True,
                        )
                    copier(evac).tensor_copy(dst[:, mc, js], ps)
                return emit

            def v_group(sc):
                def emit():
                    ps = mmp.tile([128, 512], F32, tag="mm")
                    has_b = "v" in bsb
                    for kc in range(8):
                        nc.tensor.matmul(
                            ps[:, 0:DLOC],
                            lhsT=xt["v"][:, kc, _ts(sc, 128)],
                            rhs=wsb["v"][:, kc, :],
                            start=(kc == 0), stop=(kc == 7 and not has_b),
                        )
                    if has_b:
                        nc.tensor.matmul(
                            ps[:, 0:DLOC], lhsT=ones_r[0:1, 0:128],
                            rhs=bsb["v"][0:1, :], start=False, stop=True,
                        )
                    copier(evac).tensor_copy(
                        V_sb[:, j * 4 + sc, :, 0:DH],
                        ps[:, 0:DLOC].rearrange("p (h d) -> p h d", h=HL),
                    )
                return emit

            return ([qk_group(b, d, mc) for b, d in (("q", QT_sb), ("k", KT_sb))
                     for mc in range(2)] + [v_group(sc) for sc in range(4)])

        def denorm_thunks(j, pc, O, OTs, sm):
            # softmax denominators: sums row (lane 64, staged to SBUF by the
            # caller right after the AV flush), broadcast to partitions 0:64
            # via K=1 matmul, then a single fused divide straight out of the
            # O accumulator in PSUM. Drip-fed into the NEXT attention loop so
            # the PE never blocks on the cross-engine chain.
            otpair = otp.tile([128, 512], BF16, tag=f"otp{pc}",
                              name=f"otp_{j}_{pc}")
            OTs[pc] = otpair
            st = {}

            def t_bcast():
                for i in range(2):
                    Sps = mmp.tile([128, 512], F32, tag="mm",
                                   name=f"sps_{j}_{pc}_{i}")
                    nc.tensor.matmul(
                        Sps[0:64, :], lhsT=ones64[64:65, 0:64],
                        rhs=sm[i][64:65, :],
                        start=True, stop=True, tile_position=(64, 0),
                    )
                    st[i] = Sps

            def t_rc():
                # DVE ops may read at most ONE operand from PSUM, so take the
                # reciprocal into SBUF first; the multiply then pairs the O
                # accumulator (PSUM) with rc (SBUF)
                for i in range(2):
                    rc = smallp.tile([64, 512], F32, tag="rc",
                                     name=f"rc_{j}_{pc}_{i}")
                    nc.vector.reciprocal_approx_fast(out=rc, in_=st[i][0:64, :])
                    st[i] = rc

            def t_mul0():
                nc.vector.tensor_tensor(
                    out=otpair[0:64, :], in0=O[0][0:64, :],
                    in1=st[0], op=MULT)

            def t_mul1():
                ot_tmp = smallp.tile([64, 512], BF16, tag="ott",
                                     name=f"ott_{j}_{pc}")
                nc.vector.tensor_tensor(
                    out=ot_tmp, in0=O[1][0:64, :],
                    in1=st[1], op=MULT)
                # partition shift 0:64 -> 64:128 (gpsimd can remap
                # partitions SBUF->SBUF without the DMA latency chain)
                nc.gpsimd.tensor_copy(otpair[64:128, :], ot_tmp[:, :])

            return [t_bcast, t_rc, t_mul0, t_mul1]

        def dense_thunks(j, js, OTs):
            box = {}

            def grp(mc):
                def emit():
                    if j == NSB - 1 and mc % 2:
                        # tail: the attention loops are over, so the L pool's
                        # banks are free — rotate through them too to double
                        # the dense pipeline depth
                        dps = lp.tile([128, 2, SBK], F32, tag="L",
                                      name=f"dpsl_{j}_{mc}")[:, 0, :]
                    else:
                        dps = mmp.tile([128, 512], F32, tag="mm",
                                       name=f"dps_{j}_{mc}")
                    for pc in range(2):
                        nc.tensor.matmul(
                            dps[:, :], lhsT=dn_sb[:, pc, _ts(mc, 128)],
                            rhs=OTs[pc][:, :], start=(pc == 0), stop=(pc == 1),
                        )
                    # pair consecutive mc's into one ev tile / one out-DMA
                    # (halves the 625ns-per-DMA HWDGE serialization)
                    if mc % 2 == 0:
                        box["ev"] = evp.tile([128, 2, 512], BF16, tag="ev",
                                             name=f"ev_{j}_{mc}")
                    ev = box["ev"]
                    if j == NSB - 1:
                        # tail: exps are done, ACT is free — alternate whole
                        # evacs across DVE and ACT so both queues drain the
                        # eight copies in parallel
                        if mc % 2:
                            nc.scalar.copy(out=ev[:, 1, :], in_=dps)
                        else:
                            nc.vector.tensor_copy(ev[:, 0, :], dps)
                    else:
                        copier().tensor_copy(ev[:, mc % 2, :], dps)
                    if mc % 2:
                        nc.sync.dma_start(
                            out=outT_r[:, mc - 1:mc + 1, js],
                            in_=ev)
                return emit
            return [grp(mc) for mc in range(8)]

        def phase_B(j, js, fill, lowq):
            # ---------- attention + dense for q-block j ----------
            # `fill`: queue of thunks (phase-A groups, previous block's
            # denorm + dense), one emitted per kc iteration right after the
            # logits matmuls, where the PE would otherwise wait on the exp.
            nkc = (j + 1) * 4 if causal else S // KCH
            OTs = [None, None]
            for pc in range(2):
                O = [
                    opp.tile([65, 512], F32, tag=f"o{i}", name=f"O_{j}_{pc}_{i}")
                    for i in range(2)
                ]
                pend = []  # software pipeline: AV trails logits by PD kc's

                def emit_av(kc, off, last, PT):
                    for i in range(2):
                        nc.tensor.matmul(
                            O[i][0:65, off:SBK],
                            lhsT=V_sb[:, kc, 2 * pc + i, :],
                            rhs=PT[:, i, off:SBK],
                            start=(kc == 0), stop=last,
                            skip_group_check=True,
                        )

                for kc in range(nkc):
                    diag = causal and kc >= 4 * j
                    off = (kc - 4 * j) * KCH if diag else 0
                    L = lp.tile([128, 2, SBK], F32, tag="L")
                    for i in range(2):
                        nc.tensor.matmul(
                            L[:, i, off:SBK],
                            lhsT=KT_sb[_ts(i, 64), pc, _ts(kc, KCH)],
                            rhs=QT_sb[_ts(i, 64), pc,
                                      j * SBK + off:(j + 1) * SBK],
                            start=True, stop=True,
                            tile_position=(64 * i, 0),
                        )
                    if fill and not (j == NSB - 1 and pc == 0
                                     and kc % 2 and kc > 4):
                        fill.pop(0)()
                    elif lowq and (j != 2 or kc % 2):
                        lowq.pop(0)()
                    if not causal:
                        mk = mskp.tile([128, SBK], F32, tag="mk")
                        nc.sync.dma_start(out=mk, in_=mskT[_ts(kc, KCH), js])
                        nc.vector.tensor_tensor(
                            out=L[:, :, :], in0=L[:, :, :],
                            in1=mk[:, None, :].broadcast_to([128, 2, SBK]),
                            op=ADD,
                        )
                    PT = ptp.tile([128, 2, SBK], BF16, tag="PT")
                    nc.scalar.activation(
                        out=PT[:, :, off:SBK], in_=L[:, :, off:SBK],
                        func=AF.Exp, scale=0.125)
                    if diag:
                        # triangle mask as 0/1 multiply (off the exp edge)
                        nc.vector.tensor_tensor(
                            out=PT[:, :, off:off + KCH],
                            in0=PT[:, :, off:off + KCH],
                            in1=tri01[:, None, :].broadcast_to([128, 2, KCH]),
                            op=MULT,
                        )
                    pend.append((kc, off, kc == nkc - 1, PT))
                    if len(pend) > 3:
                        emit_av(*pend.pop(0))
                for p in pend:
                    # absorb each flushed AV's exp-wait with leftover work
                    if j == NSB - 1 and fill:
                        fill.pop(0)()
                    elif j == NSB - 1 and lowq:
                        lowq.pop(0)()
                    emit_av(*p)
                # stage the sums rows to SBUF immediately (frees the O
                # accumulators' WAR hazard early; PE isn't involved)
                sm = []
                for i in range(2):
                    t = smallp.tile([65, 512], BF16, tag="sm",
                                    name=f"sm_{j}_{pc}_{i}")
                    if j == NSB - 1 and pc == 1 and i == 1:
                        # tail: ACT is still draining the last exps — put one
                        # of the two sums copies on DVE so they land together
                        nc.scalar.copy(out=t[64:65, :], in_=O[i][64:65, :])
                    else:
                        copier().tensor_copy(t[64:65, :], O[i][64:65, :])
                    sm.append(t)
                # denorm gets priority over queued dense/A thunks EXCEPT one
                # older thunk kept in front: it buys the sums-staging copies a
                # full iteration of slack before the broadcast matmul reads
                # them (the next round's first AV still unblocks early enough)
                head, rest = fill[:3], fill[3:]
                fill[:] = head + denorm_thunks(j, pc, O, OTs, sm) + rest
            # dense is latency-tolerant (ot_bufs=4 removes the tile-ring
            # coupling): hold it in a low-priority queue for iterations with
            # no other fill — that is mostly the bare stretches of B2/B3
            lowq.extend(dense_thunks(j, js, OTs))

        # schedule: A0, A1 up front (PE runway; ACT free, so evacs may use
        # it), then B(j) with A(j+2)'s groups and B(j-1)'s denorm + dense
        # drip-fed into the exp-paced attention loops.
        xts = {0: xt_pend.pop(0)}
        for j in range(NSB):
            if j + 1 not in xts and j + 1 < NSB:
                xts[j + 1] = load_x(j + 1, _ts(j + 1, SBK))
            if j < 2:
                for g in phase_A_groups(j, _ts(j, SBK), xts.pop(j),
                                        evac=("scalar", "vector")):
                    g()
                if j == 0:
                    # deprioritized loads: queued behind the first x-blocks;
                    # still well ahead of their first consumers
                    nc.sync.dma_start(out=tri01, in_=tri_in[:, :])
                    nc.sync.dma_start(
                        out=dn_sb,
                        in_=dnT.rearrange("(c p) n -> p c n", p=128))
        fill = []
        lowq = []
        for j in range(NSB):
            if j + 2 < NSB:
                # A2 feeds B0's fills; A3 waits until B1 (keeps B1 from
                # starving once its dense moved to the low-priority queue)
                fill.extend(phase_A_groups(j + 2, _ts(j + 2, SBK),
                                           xts.pop(j + 2)))
            phase_B(j, _ts(j, SBK), fill, lowq)
        while fill:
            fill.pop(0)()
        while lowq:
            lowq.pop(0)()

    nc.finalize()
    return nc


_CACHE = {}


def _get_nc(causal, with_bq, with_bk, with_bv):
    key = (causal, with_bq, with_bk, with_bv)
    if key not in _CACHE:
        _CACHE[key] = build(causal, with_bq, with_bk, with_bv)
    return _CACHE[key]


def _bf(a):
    return np.ascontiguousarray(a).astype(NPBF)


def _prep_in_maps(query, key_, value, mask2d, causal, wq_w, wk_w, wv_w, dense_w,
                  wq_b, wk_b, wv_b, with_bq, with_bk, with_bv):
    in_maps = []
    xT = {}
    for b in range(B):
        xT[b] = (_bf(query[b].T), _bf(key_[b].T), _bf(value[b].T))
    mskT = None if causal else np.ascontiguousarray(mask2d.T * np.float32(-8e9))
    for c in range(NCORES):
        b, g = divmod(c, 4)
        sl = _ts(g, DLOC)
        m = {
            "xqT": xT[b][0], "xkT": xT[b][1], "xvT": xT[b][2],
            "wqT": _bf(wq_w[sl].T),
            "wkT": _bf(wk_w[sl].T),
            "wvT": _bf(wv_w[sl].T),
            "dnT": _bf(dense_w[:, sl].T),
        }
        if not causal:
            m["mskT"] = mskT
        if with_bq:
            m["bq"] = _bf(wq_b[sl])
        if with_bk:
            m["bk"] = _bf(wk_b[sl])
        if with_bv:
            m["bv"] = _bf(wv_b[sl])
        in_maps.append(m)
    return in_maps


def _run(in_maps, causal, with_bq, with_bk, with_bv, **kw):
    nc = _get_nc(causal, with_bq, with_bk, with_bv)
    return run_bass_kernel_spmd(nc, in_maps, core_ids=list(range(NCORES)), **kw)


def kernel(query, key_, value, mask, wq_w, wq_b, wk_w, wk_b, wv_w, wv_b,
           dense_w, dense_b, _profile_kw=None):
    query = np.asarray(query, np.float32)
    key_ = np.asarray(key_, np.float32)
    value = np.asarray(value, np.float32)
    mask2d = np.asarray(mask, np.float32).reshape(S, S)
    wq_w = np.asarray(wq_w, np.float32)
    wk_w = np.asarray(wk_w, np.float32)
    wv_w = np.asarray(wv_w, np.float32)
    dense_w = np.asarray(dense_w, np.float32)
    wq_b = np.asarray(wq_b, np.float32)
    wk_b = np.asarray(wk_b, np.float32)
    wv_b = np.asarray(wv_b, np.float32)
    dense_b = np.asarray(dense_b, np.float32)

    causal = bool(np.array_equal(mask2d, np.triu(np.ones((S, S), np.float32), k=1)))
    with_bq = bool(np.any(wq_b))
    with_bk = bool(np.any(wk_b))
    with_bv = bool(np.any(wv_b))

    in_maps = _prep_in_maps(query, key_, value, mask2d, causal, wq_w, wk_w, wv_w,
                            dense_w, wq_b, wk_b, wv_b, with_bq, with_bk, with_bv)
    res = _run(in_maps, causal, with_bq, with_bk, with_bv, **(_profile_kw or {}))

    out = np.empty((B, S, D), np.float32)
    for b in range(B):
        acc = res.results[4 * b]["outT"].astype(np.float32).copy()
        for g in range(1, 4):
            acc += res.results[4 * b + g]["outT"]
        out[b] = acc.T + dense_b[None, :]
    if _profile_kw:
        return out, res
    return out
